# revision 2
# baseline (speedup 1.0000x reference)
"""3-layer GAT (arXiv-style) on 8 Trainium2 NeuronCores via Bass.

Sharding: dst-node sharding (6250 nodes/core). Node phase computes each
core's h-table slice [h | a_src-dot | a_dst-dot] rows; AllGather replicates
the table; edge phase gathers per-edge source rows (indirect DMA), builds
0/1 selection matrices from dst-locals vs iota, and does segment-softmax +
feature aggregation as PSUM-accumulated matmuls. Output: log_softmax logits.
"""
import numpy as np
import ml_dtypes

import concourse.bass as bass
import concourse.mybir as mybir
import concourse.tile as tile
from concourse.bass_utils import run_bass_kernel_spmd

# ---- problem constants (hardcoded per harness contract) ----
N = 50000
E = 800000
F_IN = 128
NEG = 0.2
BN_EPS = 1e-5
NC = 8
NS = N // NC            # 6250 nodes per core
NTILE = (NS + 127) // 128   # 49 dst tiles per core
PADN = 13 * 512         # node-phase padded slice rows (6656)
ROW1 = 520              # layer0/1 table row: h(512)+as(4)+ad(4)
ROW2 = 256              # layer2 table row: h(240)+as(6)+ad(6)+pad(4)
H12, C12 = 4, 128
H2, C2 = 6, 40
AF = mybir.ActivationFunctionType
ALU = mybir.AluOpType
dt = mybir.dt
F32, BF16, I32 = dt.float32, dt.bfloat16, dt.int32
bf = ml_dtypes.bfloat16


def _hoist_waits(nc, max_keep=1):
    n = 0
    for f in nc.m.functions:
        for bb in f.blocks:
            out, changed = [], False
            for ins in bb.instructions:
                si = getattr(ins, "sync_info", None)
                if si is not None and si.on_wait:
                    keep = 0 if (isinstance(ins, mybir.InstDMACopy)
                                 and getattr(ins, "queue", None) == "qPoolDynamic") else max_keep
                    waits = list(si.on_wait)
                    if len(waits) > keep:
                        cut = len(waits) - keep
                        for w in waits[:cut]:
                            out.append(mybir.InstEventSemaphore(
                                name=f"I-hw-{n}", engine=ins.engine, ins=[], outs=[],
                                sync_info=mybir.SyncInfo(on_wait=[w], on_update=[])))
                            n += 1
                        si.on_wait = waits[cut:]
                        changed = True
                out.append(ins)
            if changed:
                bb.instructions = out
    return n


def _edge_phase(nc, sb, ps, lay, NSUB, table, idx_t, dl_t, dlr_t, adsl, io, ioc,
                pout_next, out_final, b2r):
    """One layer's edge phase: 49 dst tiles."""
    ROW = ROW1 if lay < 2 else ROW2
    H = H12 if lay < 2 else H2
    C = C12 if lay < 2 else C2
    HC = H * C
    for t in range(NTILE):
        nreal = min(128, NS - t * 128)
        it = sb.tile([128, NSUB], I32, tag="eidx")
        nc.sync.dma_start(out=it[:], in_=idx_t[t])
        dl = sb.tile([128, NSUB], F32, tag="edl")
        nc.sync.dma_start(out=dl[:], in_=dl_t[t])
        dlr = sb.tile([128, NSUB * 128], F32, tag="edlr")
        nc.sync.dma_start(out=dlr[:], in_=dlr_t[t:t + 1, :].to_broadcast([128, NSUB * 128]))
        adt = sb.tile([128, 4 if lay < 2 else 8], BF16, tag="eadt")
        nc.sync.dma_start(out=adt[:, :H], in_=adsl[t * 128:(t + 1) * 128, :H])

        G = sb.tile([128, NSUB * ROW], BF16, tag="eG")
        for j in range(NSUB):
            nc.gpsimd.indirect_dma_start(
                out=G[:, j * ROW:(j + 1) * ROW], out_offset=None, in_=table[:],
                in_offset=bass.IndirectOffsetOnAxis(ap=it[:, j:j + 1], axis=0))

        S = sb.tile([128, NSUB * 128], BF16, tag="eS")
        nc.vector.tensor_tensor(
            out=S[:].rearrange("p (n f) -> p n f", f=128),
            in0=dl[:].rearrange("p (n o) -> p n o", o=1).to_broadcast([128, NSUB, 128]),
            in1=io[:].rearrange("p (o f) -> p o f", o=1).to_broadcast([128, NSUB, 128]),
            op=ALU.is_equal)
        ST = sb.tile([128, NSUB * 128], BF16, tag="eST")
        nc.vector.tensor_tensor(
            out=ST[:], in0=dlr[:], in1=ioc[:].to_broadcast([128, NSUB * 128]),
            op=ALU.is_equal)

        pad_ = ps.tile([128, NSUB * H], F32, space="PSUM", tag="pad")
        for j in range(NSUB):
            nc.tensor.matmul(out=pad_[:, H * j:H * (j + 1)],
                             lhsT=ST[:, j * 128:(j + 1) * 128],
                             rhs=adt[:, :H], start=True, stop=True)

        asE = sb.tile([128, NSUB * H], F32, tag="easE")
        nc.vector.tensor_copy(
            out=asE[:].rearrange("p (n f) -> p n f", f=H),
            in_=G[:].rearrange("p (n e) -> p n e", e=ROW)[:, :, HC:HC + H])
        sE = sb.tile([128, NSUB * H], F32, tag="esE")
        nc.vector.tensor_add(out=sE[:], in0=asE[:], in1=pad_[:])
        m2 = sb.tile([128, NSUB * H], F32, tag="em2")
        nc.vector.tensor_scalar(out=m2[:], in0=sE[:], scalar1=0.0, scalar2=NEG,
                                op0=ALU.min, op1=ALU.mult)
        r_ = sb.tile([128, NSUB * H], F32, tag="er")
        nc.vector.tensor_scalar_max(out=r_[:], in0=sE[:], scalar1=0.0)
        lr = sb.tile([128, NSUB * H], F32, tag="elr")
        nc.vector.tensor_add(out=lr[:], in0=r_[:], in1=m2[:])
        w = sb.tile([128, NSUB * H], F32, tag="ew")
        nc.scalar.activation(out=w[:], in_=lr[:], func=AF.Exp)
        wb = sb.tile([128, NSUB * H], BF16, tag="ewb")
        nc.vector.tensor_copy(out=wb[:], in_=w[:])

        Gw = sb.tile([128, NSUB * HC], BF16, tag="eGw")
        nc.vector.tensor_tensor(
            out=Gw[:].rearrange("p (n h c) -> p n h c", h=H, c=C),
            in0=G[:].rearrange("p (n e) -> p n e", e=ROW)[:, :, 0:HC]
                 .rearrange("p n (h c) -> p n h c", h=H),
            in1=wb[:].rearrange("p (n h o) -> p n h o", h=H, o=1).to_broadcast([128, NSUB, H, C]),
            op=ALU.mult)

        pnum = ps.tile([128, HC], F32, space="PSUM", tag="pnum")
        pden = ps.tile([128, H], F32, space="PSUM", tag="pden")
        for j in range(NSUB):
            nc.tensor.matmul(out=pnum[:], lhsT=S[:, j * 128:(j + 1) * 128],
                             rhs=Gw[:, j * HC:(j + 1) * HC],
                             start=(j == 0), stop=(j == NSUB - 1))
            nc.tensor.matmul(out=pden[:], lhsT=S[:, j * 128:(j + 1) * 128],
                             rhs=wb[:, j * H:(j + 1) * H],
                             start=(j == 0), stop=(j == NSUB - 1))
        rden = sb.tile([128, H], F32, tag="erden")
        nc.vector.reciprocal(out=rden[:], in_=pden[:])

        if lay < 2:
            ot = sb.tile([128, 512], BF16, tag="eot")
            nc.vector.tensor_tensor(
                out=ot[:].rearrange("p (h c) -> p h c", h=H),
                in0=pnum[:].rearrange("p (h c) -> p h c", h=H),
                in1=rden[:].rearrange("p (h o) -> p h o", o=1).to_broadcast([128, H, C]),
                op=ALU.mult)
            nc.sync.dma_start(out=pout_next[t * 128:t * 128 + nreal, :], in_=ot[:nreal, :])
        else:
            tmp = sb.tile([128, 240], F32, tag="etmp")
            nc.vector.tensor_tensor(
                out=tmp[:].rearrange("p (h c) -> p h c", h=H),
                in0=pnum[:].rearrange("p (h c) -> p h c", h=H),
                in1=rden[:].rearrange("p (h o) -> p h o", o=1).to_broadcast([128, H, C]),
                op=ALU.mult)
            acc = sb.tile([128, 40], F32, tag="eacc")
            nc.vector.tensor_add(out=acc[:], in0=tmp[:, 0:40], in1=tmp[:, 40:80])
            for hh in range(2, 6):
                nc.vector.tensor_add(out=acc[:], in0=acc[:], in1=tmp[:, hh * 40:(hh + 1) * 40])
            z = sb.tile([128, 40], F32, tag="ez")
            nc.vector.tensor_scalar_mul(out=z[:], in0=acc[:], scalar1=1.0 / 6.0)
            nc.vector.tensor_add(out=z[:], in0=z[:], in1=b2r[:])
            mx = sb.tile([128, 1], F32, tag="emx")
            nc.vector.reduce_max(out=mx[:], in_=z[:], axis=mybir.AxisListType.X)
            zs = sb.tile([128, 40], F32, tag="ezs")
            nc.vector.tensor_scalar(out=zs[:], in0=z[:], scalar1=mx[:, 0:1], scalar2=None,
                                    op0=ALU.subtract)
            ex = sb.tile([128, 40], F32, tag="eex")
            nc.scalar.activation(out=ex[:], in_=zs[:], func=AF.Exp)
            sm = sb.tile([128, 1], F32, tag="esm")
            nc.vector.reduce_sum(out=sm[:], in_=ex[:], axis=mybir.AxisListType.X)
            ln = sb.tile([128, 1], F32, tag="eln")
            nc.scalar.activation(out=ln[:], in_=sm[:], func=AF.Ln)
            lsm = sb.tile([128, 40], F32, tag="elsm")
            nc.vector.tensor_scalar(out=lsm[:], in0=zs[:], scalar1=ln[:, 0:1], scalar2=None,
                                    op0=ALU.subtract)
            nc.sync.dma_start(out=out_final[t * 128:t * 128 + nreal, :], in_=lsm[:nreal, :])


def _node_phase(nc, sb, ps, lay, xin, pout, wt, blk, bn_sc, bn_sh, tbl, adsl):
    """Node phase: compute table slice rows [h | as | ad] for this core's nodes."""
    if lay == 0:
        d_in, d_out, H, ROW, HC = 128, 512, 4, ROW1, 512
    elif lay == 1:
        d_in, d_out, H, ROW, HC = 512, 512, 4, ROW1, 512
    else:
        d_in, d_out, H, ROW, HC = 512, 256, 6, ROW2, 240
    NFB = d_in // 128
    for it in range(13):
        n0 = it * 512
        aT = []
        for fb in range(NFB):
            if lay == 0:
                aT.append(xin[:, n0:n0 + 512])
                continue
            a = sb.tile([128, 512], BF16, tag=f"naT{fb}")
            zb = sb.tile([128, 512], BF16, tag="nzb")
            nc.sync.dma_start(out=zb[:],
                              in_=pout[n0:n0 + 512, fb * 128:(fb + 1) * 128],
                              transpose=True)
            z = sb.tile([128, 512], F32, tag="nzf")
            nc.scalar.activation(out=z[:], in_=zb[:], func=AF.Identity,
                                 bias=bn_sh[:, fb:fb + 1], scale=bn_sc[:, fb:fb + 1])
            mm = sb.tile([128, 512], F32, tag="nmm")
            nc.vector.tensor_scalar_min(out=mm[:], in0=z[:], scalar1=0.0)
            ee = sb.tile([128, 512], F32, tag="nee")
            nc.scalar.activation(out=ee[:], in_=mm[:], func=AF.Exp)
            rr = sb.tile([128, 512], F32, tag="nrr")
            nc.vector.tensor_scalar_max(out=rr[:], in0=z[:], scalar1=0.0)
            nc.vector.tensor_add(out=rr[:], in0=rr[:], in1=ee[:])
            nc.vector.tensor_scalar(out=a[:], in0=rr[:], scalar1=-1.0, scalar2=None,
                                    op0=ALU.add)
            aT.append(a[:])
        for nb in range(4):
            ph = ps.tile([128, d_out], F32, space="PSUM", tag="nph")
            pa = ps.tile([128, 2 * H], F32, space="PSUM", tag="npa")
            for fb in range(NFB):
                lhsT = aT[fb][:, nb * 128:(nb + 1) * 128]
                nc.tensor.matmul(out=ph[:], lhsT=lhsT,
                                 rhs=wt[:, fb * blk:fb * blk + d_out],
                                 start=(fb == 0), stop=(fb == NFB - 1))
                nc.tensor.matmul(out=pa[:], lhsT=lhsT,
                                 rhs=wt[:, fb * blk + d_out:(fb + 1) * blk],
                                 start=(fb == 0), stop=(fb == NFB - 1))
            row0 = n0 + nb * 128
            nreal = min(128, max(0, NS - row0))
            if nreal == 0:
                continue
            tb = sb.tile([128, ROW], BF16, tag="ntb")
            if lay < 2:
                nc.vector.tensor_copy(out=tb[:, 0:512], in_=ph[:])
            else:
                nc.vector.tensor_copy(out=tb[:, 0:120], in_=ph[:, 0:120])
                nc.vector.tensor_copy(out=tb[:, 120:240], in_=ph[:, 128:248])
            nc.vector.tensor_copy(out=tb[:, HC:HC + H].rearrange("p (n o) -> p n o", o=1),
                                  in_=pa[:].rearrange("p (n s) -> p n s", s=2)[:, :, 0:1])
            nc.vector.tensor_copy(out=tb[:, HC + H:HC + 2 * H].rearrange("p (n o) -> p n o", o=1),
                                  in_=pa[:].rearrange("p (n s) -> p n s", s=2)[:, :, 1:2])
            nc.sync.dma_start(out=tbl[row0:row0 + nreal, :], in_=tb[:nreal, :])
            ab = sb.tile([128, 8], BF16, tag="nab")
            nc.vector.tensor_copy(out=ab[:, :H].rearrange("p (n o) -> p n o", o=1),
                                  in_=pa[:].rearrange("p (n s) -> p n s", s=2)[:, :, 1:2])
            nc.sync.dma_start(out=adsl[row0:row0 + nreal, :H], in_=ab[:nreal, :H])


def _build(NSUB, NSUB2):
    nc = bass.Bass()
    # ---- inputs (per-core) ----
    xT = nc.declare_dram_parameter("xT", [128, PADN], BF16, isOutput=False)
    W0 = nc.declare_dram_parameter("W0p", [128, 512 + 8], BF16, isOutput=False)
    W1 = nc.declare_dram_parameter("W1p", [128, 4 * (512 + 8)], BF16, isOutput=False)
    W2 = nc.declare_dram_parameter("W2p", [128, 4 * (256 + 12)], BF16, isOutput=False)
    bnsc0 = nc.declare_dram_parameter("bnsc0", [128, 4], F32, isOutput=False)
    bnsh0 = nc.declare_dram_parameter("bnsh0", [128, 4], F32, isOutput=False)
    bnsc1 = nc.declare_dram_parameter("bnsc1", [128, 4], F32, isOutput=False)
    bnsh1 = nc.declare_dram_parameter("bnsh1", [128, 4], F32, isOutput=False)
    b2r = nc.declare_dram_parameter("b2r", [128, 40], F32, isOutput=False)
    iota = nc.declare_dram_parameter("iota", [128, 128], F32, isOutput=False)
    iotac = nc.declare_dram_parameter("iotac", [128, 1], F32, isOutput=False)
    eidx = nc.declare_dram_parameter("eidx", [NTILE, 128, NSUB], I32, isOutput=False)
    edl = nc.declare_dram_parameter("edl", [NTILE, 128, NSUB], F32, isOutput=False)
    edlr = nc.declare_dram_parameter("edlr", [NTILE, NSUB * 128], F32, isOutput=False)
    eidx2 = nc.declare_dram_parameter("eidx2", [NTILE, 128, NSUB2], I32, isOutput=False)
    edl2 = nc.declare_dram_parameter("edl2", [NTILE, 128, NSUB2], F32, isOutput=False)
    edlr2 = nc.declare_dram_parameter("edlr2", [NTILE, NSUB2 * 128], F32, isOutput=False)
    out = nc.declare_dram_parameter("out", [NS, 40], F32, isOutput=True)
    # ---- internal ----
    tbl0 = nc.dram_tensor("tbl0", [NS, ROW1], BF16)
    tbl1 = nc.dram_tensor("tbl1", [NS, ROW1], BF16)
    tbl2 = nc.dram_tensor("tbl2", [NS, ROW2], BF16)
    tab0 = nc.dram_tensor("tab0", [N, ROW1], BF16, addr_space="Shared")
    tab1 = nc.dram_tensor("tab1", [N, ROW1], BF16, addr_space="Shared")
    tab2 = nc.dram_tensor("tab2", [N, ROW2], BF16, addr_space="Shared")
    pout0 = nc.dram_tensor("pout0", [PADN, 512], BF16)
    pout1 = nc.dram_tensor("pout1", [PADN, 512], BF16)
    adsl0 = nc.dram_tensor("adsl0", [NTILE * 128, 8], BF16)
    adsl1 = nc.dram_tensor("adsl1", [NTILE * 128, 8], BF16)
    adsl2 = nc.dram_tensor("adsl2", [NTILE * 128, 8], BF16)

    rg = [list(range(NC))]
    with tile.TileContext(nc) as tc:
        with tc.tile_pool(name="cst", bufs=1) as cst, \
             tc.tile_pool(name="sb", bufs=2) as sb, \
             tc.tile_pool(name="ps", bufs=1, space="PSUM") as ps:
            io = cst.tile([128, 128], F32)
            nc.sync.dma_start(out=io[:], in_=iota[:])
            ioc = cst.tile([128, 1], F32)
            nc.sync.dma_start(out=ioc[:], in_=iotac[:])
            b2 = cst.tile([128, 40], F32)
            nc.sync.dma_start(out=b2[:], in_=b2r[:])
            xt = cst.tile([128, PADN], BF16)
            nc.sync.dma_start(out=xt[:], in_=xT[:])
            w0 = cst.tile([128, 520], BF16)
            nc.sync.dma_start(out=w0[:], in_=W0[:])
            w1 = cst.tile([128, 4 * 520], BF16)
            nc.sync.dma_start(out=w1[:], in_=W1[:])
            w2 = cst.tile([128, 4 * 268], BF16)
            nc.sync.dma_start(out=w2[:], in_=W2[:])
            sc0 = cst.tile([128, 4], F32); nc.sync.dma_start(out=sc0[:], in_=bnsc0[:])
            sh0 = cst.tile([128, 4], F32); nc.sync.dma_start(out=sh0[:], in_=bnsh0[:])
            sc1 = cst.tile([128, 4], F32); nc.sync.dma_start(out=sc1[:], in_=bnsc1[:])
            sh1 = cst.tile([128, 4], F32); nc.sync.dma_start(out=sh1[:], in_=bnsh1[:])
            zz = cst.tile([128, 512], BF16)
            nc.gpsimd.memset(zz[:], 0.0)
            # zero pad rows of pout0/pout1 (rows NS..PADN)
            for pout in (pout0, pout1):
                r = NS
                while r < PADN:
                    k = min(128, PADN - r)
                    nc.sync.dma_start(out=pout[r:r + k, :], in_=zz[:k, :])
                    r += k
            for adsl in (adsl0, adsl1, adsl2):
                nc.sync.dma_start(out=adsl[NS:NTILE * 128, :], in_=zz[:NTILE * 128 - NS, :8])

            # Layer 0
            _node_phase(nc, sb, ps, 0, xt[:], None, w0[:], 520, None, None, tbl0, adsl0)
            nc.gpsimd.collective_compute("AllGather", ALU.bypass, replica_groups=rg,
                                         ins=[tbl0[:]], outs=[tab0[:]])
            _edge_phase(nc, sb, ps, 0, NSUB, tab0, eidx, edl, edlr, adsl0, io, ioc,
                        pout0, None, None)
            # Layer 1
            _node_phase(nc, sb, ps, 1, None, pout0, w1[:], 520, sc0[:], sh0[:], tbl1, adsl1)
            nc.gpsimd.collective_compute("AllGather", ALU.bypass, replica_groups=rg,
                                         ins=[tbl1[:]], outs=[tab1[:]])
            _edge_phase(nc, sb, ps, 1, NSUB, tab1, eidx, edl, edlr, adsl1, io, ioc,
                        pout1, None, None)
            # Layer 2
            _node_phase(nc, sb, ps, 2, None, pout1, w2[:], 268, sc1[:], sh1[:], tbl2, adsl2)
            nc.gpsimd.collective_compute("AllGather", ALU.bypass, replica_groups=rg,
                                         ins=[tbl2[:]], outs=[tab2[:]])
            _edge_phase(nc, sb, ps, 2, NSUB2, tab2, eidx2, edl2, edlr2, adsl2, io, ioc,
                        None, out, b2[:])
    _hoist_waits(nc)
    return nc


def _prep_edges(edge_index):
    """Per-core edge arrays. Returns (NSUB, per-core dict lists)."""
    src = edge_index[0].astype(np.int64)
    dst = edge_index[1].astype(np.int64)
    loops = np.arange(N, dtype=np.int64)
    src = np.concatenate([src, loops])
    dst = np.concatenate([dst, loops])
    core = dst // NS
    tloc = (dst % NS) // 128
    per_core = []
    maxsub = 0
    for c in range(NC):
        m = core == c
        s_c, d_c, t_c = src[m], dst[m], tloc[m]
        tiles = []
        for t in range(NTILE):
            mt = t_c == t
            ss = s_c[mt]
            dd = (d_c[mt] % NS) - t * 128
            tiles.append((ss, dd))
            maxsub = max(maxsub, (len(ss) + 127) // 128)
        per_core.append(tiles)
    NSUB = maxsub
    arrs = []
    for c in range(NC):
        idx = np.zeros((NTILE, 128, NSUB), np.int32)
        dl = np.full((NTILE, 128, NSUB), 999.0, np.float32)
        for t, (ss, dd) in enumerate(per_core[c]):
            n = len(ss)
            e = np.arange(n)
            idx[t, e % 128, e // 128] = ss.astype(np.int32)
            dl[t, e % 128, e // 128] = dd.astype(np.float32)
        dlr = dl.transpose(0, 2, 1).reshape(NTILE, NSUB * 128).copy()
        arrs.append((idx, dl, dlr))
    return NSUB, arrs


def kernel(x, edge_index, W0, as0, ad0, b0, g0, bt0, m0, v0,
           W1, as1, ad1, b1, g1, bt1, m1, v1,
           W2, as2, ad2, b2):
    x = np.asarray(x, np.float32)
    NSUB, earrs = _prep_edges(np.asarray(edge_index))

    # host-side packing
    def packW(W, a_s, a_d, d_out_pad, Hh, Cc, headpad):
        # W: [d_in, d_out]; returns [128, NFB*(d_out_pad + 2H)] bf16
        d_in = W.shape[0]
        NFB = d_in // 128
        if headpad:  # L2: pad head layout (3 heads of 40 per 128-block)
            Wp = np.zeros((d_in, 256), np.float32)
            Wp[:, 0:120] = W[:, 0:120]
            Wp[:, 128:248] = W[:, 120:240]
        else:
            Wp = W.astype(np.float32)
        Wa = np.zeros((d_in, 2 * Hh), np.float32)
        for h in range(Hh):
            Wa[:, 2 * h] = W[:, h * Cc:(h + 1) * Cc] @ a_s[h]
            Wa[:, 2 * h + 1] = W[:, h * Cc:(h + 1) * Cc] @ a_d[h]
        blk = np.concatenate([Wp, Wa], axis=1)          # [d_in, d_out_pad+2H]
        blk = blk.reshape(NFB, 128, -1)
        return np.concatenate([blk[i] for i in range(NFB)], axis=1).astype(bf)

    W0 = np.asarray(W0, np.float32); W1 = np.asarray(W1, np.float32); W2 = np.asarray(W2, np.float32)
    as0 = np.asarray(as0, np.float32); ad0 = np.asarray(ad0, np.float32)
    as1 = np.asarray(as1, np.float32); ad1 = np.asarray(ad1, np.float32)
    as2 = np.asarray(as2, np.float32); ad2 = np.asarray(ad2, np.float32)
    W0p = packW(W0, as0, ad0, 512, 4, 128, False)
    W1p = packW(W1, as1, ad1, 512, 4, 128, False)
    W2p = packW(W2, as2, ad2, 256, 6, 40, True)

    def bnfold(g, bt, m, v, b):
        sc = np.asarray(g, np.float32) / np.sqrt(np.asarray(v, np.float32) + BN_EPS)
        sh = (np.asarray(b, np.float32) - np.asarray(m, np.float32)) * sc + np.asarray(bt, np.float32)
        return sc.reshape(4, 128).T.copy(), sh.reshape(4, 128).T.copy()
    sc0, sh0 = bnfold(g0, bt0, m0, v0, b0)
    sc1, sh1 = bnfold(g1, bt1, m1, v1, b1)
    b2rep = np.broadcast_to(np.asarray(b2, np.float32), (128, 40)).copy()

    iota = np.broadcast_to(np.arange(128, dtype=np.float32), (128, 128)).copy()
    iotac = np.arange(128, dtype=np.float32)[:, None].copy()

    xT = x.T.astype(bf)   # [128, N]
    in_maps = []
    for c in range(NC):
        xts = np.zeros((128, PADN), bf)
        xts[:, :NS] = xT[:, c * NS:(c + 1) * NS]
        idx, dl, dlr = earrs[c]
        in_maps.append({
            "xT": xts, "W0p": W0p, "W1p": W1p, "W2p": W2p,
            "bnsc0": sc0, "bnsh0": sh0, "bnsc1": sc1, "bnsh1": sh1,
            "b2r": b2rep, "iota": iota, "iotac": iotac,
            "eidx": idx, "edl": dl, "edlr": dlr,
            "eidx2": idx, "edl2": dl, "edlr2": dlr,
        })

    global _last_in_maps, _last_nsub
    _last_in_maps = in_maps
    _last_nsub = NSUB
    nc = _build(NSUB, NSUB)
    res = run_bass_kernel_spmd(nc, in_maps, core_ids=list(range(NC)))
    out = np.concatenate([res.results[c]["out"] for c in range(NC)], axis=0)
    return out.astype(np.float32)



# revision 13
# speedup vs baseline: 1.1178x; 1.1178x over previous
"""3-layer GAT (arXiv-style) on 8 Trainium2 NeuronCores via Bass.

Sharding: dst-node sharding (6250 nodes/core). Node phase computes each
core's h-table slice [h | a_src-dot] rows, split at local row 3072 into two
tables; two AllGathers replicate them (each <32768 rows so dma_gather's
int16 indices reach every row). Edge phase gathers per-edge source rows
with gpsimd dma_gather (<=1024 indices/call), builds 0/1 selection
matrices from dst-locals vs iota, and does segment-softmax + feature
aggregation as PSUM-accumulated matmuls. Output: log_softmax logits.
"""
import numpy as np
import ml_dtypes

import concourse.bass as bass
import concourse.mybir as mybir
import concourse.tile as tile
from concourse.bass_utils import run_bass_kernel_spmd
from concourse.library_config import all_libraries, standard
import bass_rust

# ---- problem constants (hardcoded per harness contract) ----
N = 50000
E = 800000
F_IN = 128
NEG = 0.2
BN_EPS = 1e-5
NC = 8
NS = N // NC            # 6250 nodes per core
NTILE = (NS + 127) // 128   # 49 dst tiles per core
PADN = 13 * 512         # node-phase padded slice rows (6656)
SPL = 3072              # local-row split: tblA rows [0,3072), tblB [3072,6250)
NSB = NS - SPL          # 3178 rows per core in tblB
ROW1 = 640              # layer0/1 table row: h(512)+as(4)+pad (1280B, %256)
ROW2 = 256              # layer2 table row: h(240)+as(6)+ad(6)+pad (512B, %256)
H12, C12 = 4, 128
H2, C2 = 6, 40
MAXIDX = 1024           # dma_gather per-call index cap (HW ring limit ~1k)
AF = mybir.ActivationFunctionType
ALU = mybir.AluOpType
dt = mybir.dt
F32, BF16, I16 = dt.float32, dt.bfloat16, dt.int16
bf = ml_dtypes.bfloat16


def _hoist_waits(nc, max_keep=1):
    n = 0
    for f in nc.m.functions:
        for bb in f.blocks:
            out, changed = [], False
            for ins in bb.instructions:
                si = getattr(ins, "sync_info", None)
                if si is not None and si.on_wait:
                    keep = 0 if (isinstance(ins, mybir.InstDMAGatherAnt)
                                 or (isinstance(ins, mybir.InstDMACopy)
                                     and getattr(ins, "queue", None) == "qPoolDynamic")) else max_keep
                    waits = list(si.on_wait)
                    if len(waits) > keep:
                        cut = len(waits) - keep
                        for w in waits[:cut]:
                            out.append(mybir.InstEventSemaphore(
                                name=f"I-hw-{n}", engine=ins.engine, ins=[], outs=[],
                                sync_info=mybir.SyncInfo(on_wait=[w], on_update=[])))
                            n += 1
                        si.on_wait = waits[cut:]
                        changed = True
                out.append(ins)
            if changed:
                bb.instructions = out
    return n


def _call_widths(jseg):
    """Split a segment of jseg 128-idx columns into balanced calls of <=8."""
    ncalls = -(-jseg // (MAXIDX // 128))
    base = jseg // ncalls
    rem = jseg - base * ncalls
    return [base + (1 if i < rem else 0) for i in range(ncalls)]


def _edge_phase(nc, sb, ps, lay, J1, J2, tabA, tabB, idx_t, dl_t, dlr_t, adsl,
                io, ioc, pout_next, out_final, b2r, wregs):
    """One layer's edge phase: 49 dst tiles."""
    ROW = ROW1 if lay < 2 else ROW2
    H = H12 if lay < 2 else H2
    C = C12 if lay < 2 else C2
    HC = H * C
    NSUB = J1 + J2
    S16 = NSUB * 8
    for t in range(NTILE):
        nreal = min(128, NS - t * 128)
        it = sb.tile([128, S16], I16, tag="eidx")
        nc.sync.dma_start(out=it[:], in_=idx_t[t])
        dl = sb.tile([128, NSUB], BF16, tag="edl")
        nc.sync.dma_start(out=dl[:], in_=dl_t[t])
        dlr = sb.tile([128, NSUB * 128], BF16, tag="edlr")
        nc.sync.dma_start(out=dlr[:], in_=dlr_t[t:t + 1, :].to_broadcast([128, NSUB * 128]))
        adt = sb.tile([128, 4 if lay < 2 else 8], BF16, tag="eadt")
        nc.sync.dma_start(out=adt[:, :H], in_=adsl[t * 128:(t + 1) * 128, :H])

        G = sb.tile([128, NSUB * ROW], BF16, tag="eG")
        col = 0
        for tb_, jseg in ((tabA, J1), (tabB, J2)):
            for w in _call_widths(jseg):
                nc.gpsimd.dma_gather(
                    G[:, col * ROW:(col + w) * ROW].rearrange("p (j r) -> p j r", r=ROW),
                    tb_[:], it[:, col * 8:(col + w) * 8], w * 128, wregs[w], ROW)
                col += w

        S = sb.tile([128, NSUB * 128], BF16, tag="eS")
        nc.vector.tensor_tensor(
            out=S[:].rearrange("p (n f) -> p n f", f=128),
            in0=dl[:].rearrange("p (n o) -> p n o", o=1).to_broadcast([128, NSUB, 128]),
            in1=io[:].rearrange("p (o f) -> p o f", o=1).to_broadcast([128, NSUB, 128]),
            op=ALU.is_equal)
        ST = sb.tile([128, NSUB * 128], BF16, tag="eST")
        nc.vector.tensor_tensor(
            out=ST[:], in0=dlr[:], in1=ioc[:].to_broadcast([128, NSUB * 128]),
            op=ALU.is_equal)

        pad_ = ps.tile([128, NSUB * H], F32, space="PSUM", tag="pad")
        for j in range(NSUB):
            nc.tensor.matmul(out=pad_[:, H * j:H * (j + 1)],
                             lhsT=ST[:, j * 128:(j + 1) * 128],
                             rhs=adt[:, :H], start=True, stop=True)

        asE = sb.tile([128, NSUB * H], F32, tag="easE")
        nc.vector.tensor_copy(
            out=asE[:].rearrange("p (n f) -> p n f", f=H),
            in_=G[:].rearrange("p (n e) -> p n e", e=ROW)[:, :, HC:HC + H])
        sE = sb.tile([128, NSUB * H], F32, tag="esE")
        nc.vector.tensor_add(out=sE[:], in0=asE[:], in1=pad_[:])
        m2 = sb.tile([128, NSUB * H], F32, tag="em2")
        nc.vector.tensor_scalar(out=m2[:], in0=sE[:], scalar1=0.0, scalar2=NEG,
                                op0=ALU.min, op1=ALU.mult)
        r_ = sb.tile([128, NSUB * H], F32, tag="er")
        nc.vector.tensor_scalar_max(out=r_[:], in0=sE[:], scalar1=0.0)
        lr = sb.tile([128, NSUB * H], F32, tag="elr")
        nc.vector.tensor_add(out=lr[:], in0=r_[:], in1=m2[:])
        w_ = sb.tile([128, NSUB * H], F32, tag="ew")
        nc.scalar.activation(out=w_[:], in_=lr[:], func=AF.Exp)
        wb = sb.tile([128, NSUB * H], BF16, tag="ewb")
        nc.vector.tensor_copy(out=wb[:], in_=w_[:])

        Gw = sb.tile([128, NSUB * HC], BF16, tag="eGw")
        nc.vector.tensor_tensor(
            out=Gw[:].rearrange("p (n h c) -> p n h c", h=H, c=C),
            in0=G[:].rearrange("p (n e) -> p n e", e=ROW)[:, :, 0:HC]
                 .rearrange("p n (h c) -> p n h c", h=H),
            in1=wb[:].rearrange("p (n h o) -> p n h o", h=H, o=1).to_broadcast([128, NSUB, H, C]),
            op=ALU.mult)

        pnum = ps.tile([128, HC], F32, space="PSUM", tag="pnum")
        pden = ps.tile([128, H], F32, space="PSUM", tag="pden")
        for j in range(NSUB):
            nc.tensor.matmul(out=pnum[:], lhsT=S[:, j * 128:(j + 1) * 128],
                             rhs=Gw[:, j * HC:(j + 1) * HC],
                             start=(j == 0), stop=(j == NSUB - 1))
            nc.tensor.matmul(out=pden[:], lhsT=S[:, j * 128:(j + 1) * 128],
                             rhs=wb[:, j * H:(j + 1) * H],
                             start=(j == 0), stop=(j == NSUB - 1))
        rden = sb.tile([128, H], F32, tag="erden")
        nc.vector.reciprocal(out=rden[:], in_=pden[:])

        if lay < 2:
            ot = sb.tile([128, 512], BF16, tag="eot")
            for hh in range(H):
                nc.scalar.activation(out=ot[:, hh * C:(hh + 1) * C],
                                     in_=pnum[:, hh * C:(hh + 1) * C],
                                     func=AF.Identity, scale=rden[:, hh:hh + 1])
            nc.sync.dma_start(out=pout_next[t * 128:t * 128 + nreal, :], in_=ot[:nreal, :])
        else:
            tmp = sb.tile([128, 240], F32, tag="etmp")
            for hh in range(H):
                nc.scalar.activation(out=tmp[:, hh * C:(hh + 1) * C],
                                     in_=pnum[:, hh * C:(hh + 1) * C],
                                     func=AF.Identity, scale=rden[:, hh:hh + 1])
            acc = sb.tile([128, 40], F32, tag="eacc")
            nc.vector.tensor_add(out=acc[:], in0=tmp[:, 0:40], in1=tmp[:, 40:80])
            for hh in range(2, 6):
                nc.vector.tensor_add(out=acc[:], in0=acc[:], in1=tmp[:, hh * 40:(hh + 1) * 40])
            z = sb.tile([128, 40], F32, tag="ez")
            nc.vector.tensor_scalar_mul(out=z[:], in0=acc[:], scalar1=1.0 / 6.0)
            nc.vector.tensor_add(out=z[:], in0=z[:], in1=b2r[:])
            mx = sb.tile([128, 1], F32, tag="emx")
            nc.vector.reduce_max(out=mx[:], in_=z[:], axis=mybir.AxisListType.X)
            zs = sb.tile([128, 40], F32, tag="ezs")
            nc.vector.tensor_scalar(out=zs[:], in0=z[:], scalar1=mx[:, 0:1], scalar2=None,
                                    op0=ALU.subtract)
            ex = sb.tile([128, 40], F32, tag="eex")
            nc.scalar.activation(out=ex[:], in_=zs[:], func=AF.Exp)
            sm = sb.tile([128, 1], F32, tag="esm")
            nc.vector.reduce_sum(out=sm[:], in_=ex[:], axis=mybir.AxisListType.X)
            ln = sb.tile([128, 1], F32, tag="eln")
            nc.scalar.activation(out=ln[:], in_=sm[:], func=AF.Ln)
            lsm = sb.tile([128, 40], F32, tag="elsm")
            nc.vector.tensor_scalar(out=lsm[:], in0=zs[:], scalar1=ln[:, 0:1], scalar2=None,
                                    op0=ALU.subtract)
            nc.sync.dma_start(out=out_final[t * 128:t * 128 + nreal, :], in_=lsm[:nreal, :])


def _node_phase(nc, sb, ps, lay, xin, pout, wt, blk, bn_sc, bn_sh, tblA, tblB, adsl):
    """Node phase: compute table slice rows [h | as] for this core's nodes."""
    if lay == 0:
        d_in, d_out, H, ROW, HC = 128, 512, 4, ROW1, 512
    elif lay == 1:
        d_in, d_out, H, ROW, HC = 512, 512, 4, ROW1, 512
    else:
        d_in, d_out, H, ROW, HC = 512, 256, 6, ROW2, 240
    NFB = d_in // 128
    for it in range(13):
        n0 = it * 512
        aT = []
        for fb in range(NFB):
            if lay == 0:
                aT.append(xin[:, n0:n0 + 512])
                continue
            a = sb.tile([128, 512], BF16, tag=f"naT{fb}")
            zb = sb.tile([128, 512], BF16, tag="nzb")
            nc.sync.dma_start(out=zb[:],
                              in_=pout[n0:n0 + 512, fb * 128:(fb + 1) * 128],
                              transpose=True)
            z = sb.tile([128, 512], F32, tag="nzf")
            nc.scalar.activation(out=z[:], in_=zb[:], func=AF.Identity,
                                 bias=bn_sh[:, fb:fb + 1], scale=bn_sc[:, fb:fb + 1])
            mm = sb.tile([128, 512], F32, tag="nmm")
            nc.vector.tensor_scalar_min(out=mm[:], in0=z[:], scalar1=0.0)
            ee = sb.tile([128, 512], F32, tag="nee")
            nc.scalar.activation(out=ee[:], in_=mm[:], func=AF.Exp)
            rr = sb.tile([128, 512], F32, tag="nrr")
            nc.vector.tensor_scalar_max(out=rr[:], in0=z[:], scalar1=0.0)
            nc.vector.tensor_add(out=rr[:], in0=rr[:], in1=ee[:])
            nc.vector.tensor_scalar(out=a[:], in0=rr[:], scalar1=-1.0, scalar2=None,
                                    op0=ALU.add)
            aT.append(a[:])
        for nb in range(4):
            ph = ps.tile([128, d_out], F32, space="PSUM", tag="nph")
            pa = ps.tile([128, 2 * H], F32, space="PSUM", tag="npa")
            for fb in range(NFB):
                lhsT = aT[fb][:, nb * 128:(nb + 1) * 128]
                nc.tensor.matmul(out=ph[:], lhsT=lhsT,
                                 rhs=wt[:, fb * blk:fb * blk + d_out],
                                 start=(fb == 0), stop=(fb == NFB - 1))
                nc.tensor.matmul(out=pa[:], lhsT=lhsT,
                                 rhs=wt[:, fb * blk + d_out:(fb + 1) * blk],
                                 start=(fb == 0), stop=(fb == NFB - 1))
            row0 = n0 + nb * 128
            nreal = min(128, max(0, NS - row0))
            if nreal == 0:
                continue
            used = HC + H if lay < 2 else ROW2
            tb = sb.tile([128, used], BF16, tag="ntb")
            if lay < 2:
                nc.vector.tensor_copy(out=tb[:, 0:512], in_=ph[:])
            else:
                nc.vector.tensor_copy(out=tb[:, 0:120], in_=ph[:, 0:120])
                nc.vector.tensor_copy(out=tb[:, 120:240], in_=ph[:, 128:248])
            nc.vector.tensor_copy(out=tb[:, HC:HC + H].rearrange("p (n o) -> p n o", o=1),
                                  in_=pa[:].rearrange("p (n s) -> p n s", s=2)[:, :, 0:1])
            if lay == 2:
                nc.vector.tensor_copy(
                    out=tb[:, HC + H:HC + 2 * H].rearrange("p (n o) -> p n o", o=1),
                    in_=pa[:].rearrange("p (n s) -> p n s", s=2)[:, :, 1:2])
            if row0 < SPL:
                nc.sync.dma_start(out=tblA[row0:row0 + nreal, 0:used], in_=tb[:nreal, :])
            else:
                nc.sync.dma_start(out=tblB[row0 - SPL:row0 - SPL + nreal, 0:used],
                                  in_=tb[:nreal, :])
            ab = sb.tile([128, 8], BF16, tag="nab")
            nc.vector.tensor_copy(out=ab[:, :H].rearrange("p (n o) -> p n o", o=1),
                                  in_=pa[:].rearrange("p (n s) -> p n s", s=2)[:, :, 1:2])
            nc.sync.dma_start(out=adsl[row0:row0 + nreal, :H], in_=ab[:nreal, :H])


def _build(J1, J2):
    NSUB = J1 + J2
    S16 = NSUB * 8
    nc = bass.Bass()
    # ---- inputs (per-core) ----
    xT = nc.declare_dram_parameter("xT", [128, PADN], BF16, isOutput=False)
    W0 = nc.declare_dram_parameter("W0p", [128, 512 + 8], BF16, isOutput=False)
    W1 = nc.declare_dram_parameter("W1p", [128, 4 * (512 + 8)], BF16, isOutput=False)
    W2 = nc.declare_dram_parameter("W2p", [128, 4 * (256 + 12)], BF16, isOutput=False)
    bnsc0 = nc.declare_dram_parameter("bnsc0", [128, 4], F32, isOutput=False)
    bnsh0 = nc.declare_dram_parameter("bnsh0", [128, 4], F32, isOutput=False)
    bnsc1 = nc.declare_dram_parameter("bnsc1", [128, 4], F32, isOutput=False)
    bnsh1 = nc.declare_dram_parameter("bnsh1", [128, 4], F32, isOutput=False)
    b2r = nc.declare_dram_parameter("b2r", [128, 40], F32, isOutput=False)
    iota = nc.declare_dram_parameter("iota", [128, 128], BF16, isOutput=False)
    iotac = nc.declare_dram_parameter("iotac", [128, 1], BF16, isOutput=False)
    eidx = nc.declare_dram_parameter("eidx", [NTILE, 128, S16], I16, isOutput=False)
    edl = nc.declare_dram_parameter("edl", [NTILE, 128, NSUB], BF16, isOutput=False)
    edlr = nc.declare_dram_parameter("edlr", [NTILE, NSUB * 128], BF16, isOutput=False)
    out = nc.declare_dram_parameter("out", [NS, 40], F32, isOutput=True)
    # ---- internal ----
    tblA0 = nc.dram_tensor("tblA0", [SPL, ROW1], BF16)
    tblB0 = nc.dram_tensor("tblB0", [NSB, ROW1], BF16)
    tblA1 = nc.dram_tensor("tblA1", [SPL, ROW1], BF16)
    tblB1 = nc.dram_tensor("tblB1", [NSB, ROW1], BF16)
    tblA2 = nc.dram_tensor("tblA2", [SPL, ROW2], BF16)
    tblB2 = nc.dram_tensor("tblB2", [NSB, ROW2], BF16)
    tabA0 = nc.dram_tensor("tabA0", [NC * SPL, ROW1], BF16, addr_space="Shared")
    tabB0 = nc.dram_tensor("tabB0", [NC * NSB, ROW1], BF16, addr_space="Shared")
    tabA1 = nc.dram_tensor("tabA1", [NC * SPL, ROW1], BF16, addr_space="Shared")
    tabB1 = nc.dram_tensor("tabB1", [NC * NSB, ROW1], BF16, addr_space="Shared")
    tabA2 = nc.dram_tensor("tabA2", [NC * SPL, ROW2], BF16, addr_space="Shared")
    tabB2 = nc.dram_tensor("tabB2", [NC * NSB, ROW2], BF16, addr_space="Shared")
    pout0 = nc.dram_tensor("pout0", [PADN, 512], BF16)
    pout1 = nc.dram_tensor("pout1", [PADN, 512], BF16)
    adsl0 = nc.dram_tensor("adsl0", [NTILE * 128, 8], BF16)
    adsl1 = nc.dram_tensor("adsl1", [NTILE * 128, 8], BF16)
    adsl2 = nc.dram_tensor("adsl2", [NTILE * 128, 8], BF16)

    rg = [list(range(NC))]
    with tile.TileContext(nc) as tc:
        with tc.tile_pool(name="cst", bufs=1) as cst, \
             tc.tile_pool(name="sb", bufs=2) as sb, \
             tc.tile_pool(name="ps", bufs=1, space="PSUM") as ps:
            io = cst.tile([128, 128], BF16)
            nc.sync.dma_start(out=io[:], in_=iota[:])
            ioc = cst.tile([128, 1], BF16)
            nc.sync.dma_start(out=ioc[:], in_=iotac[:])
            b2 = cst.tile([128, 40], F32)
            nc.sync.dma_start(out=b2[:], in_=b2r[:])
            xt = cst.tile([128, PADN], BF16)
            nc.sync.dma_start(out=xt[:], in_=xT[:])
            w0 = cst.tile([128, 520], BF16)
            nc.sync.dma_start(out=w0[:], in_=W0[:])
            w1 = cst.tile([128, 4 * 520], BF16)
            nc.sync.dma_start(out=w1[:], in_=W1[:])
            w2 = cst.tile([128, 4 * 268], BF16)
            nc.sync.dma_start(out=w2[:], in_=W2[:])
            sc0 = cst.tile([128, 4], F32); nc.sync.dma_start(out=sc0[:], in_=bnsc0[:])
            sh0 = cst.tile([128, 4], F32); nc.sync.dma_start(out=sh0[:], in_=bnsh0[:])
            sc1 = cst.tile([128, 4], F32); nc.sync.dma_start(out=sc1[:], in_=bnsc1[:])
            sh1 = cst.tile([128, 4], F32); nc.sync.dma_start(out=sh1[:], in_=bnsh1[:])
            zz = cst.tile([128, 512], BF16)
            nc.gpsimd.memset(zz[:], 0.0)
            wregs = {w: nc.gpsimd.to_reg(w * 128)
                     for w in set(_call_widths(J1)) | set(_call_widths(J2))}
            # zero pad rows of pout0/pout1 (rows NS..PADN)
            for pout in (pout0, pout1):
                r = NS
                while r < PADN:
                    k = min(128, PADN - r)
                    nc.sync.dma_start(out=pout[r:r + k, :], in_=zz[:k, :])
                    r += k
            for adsl in (adsl0, adsl1, adsl2):
                nc.sync.dma_start(out=adsl[NS:NTILE * 128, :], in_=zz[:NTILE * 128 - NS, :8])

            # Layer 0
            _node_phase(nc, sb, ps, 0, xt[:], None, w0[:], 520, None, None,
                        tblA0, tblB0, adsl0)
            nc.gpsimd.collective_compute("AllGather", ALU.bypass, replica_groups=rg,
                                         ins=[tblA0[:]], outs=[tabA0[:]])
            nc.gpsimd.collective_compute("AllGather", ALU.bypass, replica_groups=rg,
                                         ins=[tblB0[:]], outs=[tabB0[:]])
            _edge_phase(nc, sb, ps, 0, J1, J2, tabA0, tabB0, eidx, edl, edlr, adsl0,
                        io, ioc, pout0, None, None, wregs)
            # Layer 1
            _node_phase(nc, sb, ps, 1, None, pout0, w1[:], 520, sc0[:], sh0[:],
                        tblA1, tblB1, adsl1)
            nc.gpsimd.collective_compute("AllGather", ALU.bypass, replica_groups=rg,
                                         ins=[tblA1[:]], outs=[tabA1[:]])
            nc.gpsimd.collective_compute("AllGather", ALU.bypass, replica_groups=rg,
                                         ins=[tblB1[:]], outs=[tabB1[:]])
            _edge_phase(nc, sb, ps, 1, J1, J2, tabA1, tabB1, eidx, edl, edlr, adsl1,
                        io, ioc, pout1, None, None, wregs)
            # Layer 2
            _node_phase(nc, sb, ps, 2, None, pout1, w2[:], 268, sc1[:], sh1[:],
                        tblA2, tblB2, adsl2)
            nc.gpsimd.collective_compute("AllGather", ALU.bypass, replica_groups=rg,
                                         ins=[tblA2[:]], outs=[tabA2[:]])
            nc.gpsimd.collective_compute("AllGather", ALU.bypass, replica_groups=rg,
                                         ins=[tblB2[:]], outs=[tabB2[:]])
            _edge_phase(nc, sb, ps, 2, J1, J2, tabA2, tabB2, eidx, edl, edlr, adsl2,
                        io, ioc, None, out, b2[:], wregs)
    _hoist_waits(nc)
    mask = {}
    for lib in all_libraries:
        for ty in lib.instructions:
            mask[ty] = mask.get(ty, 0) | (1 << lib.index)
    bass_rust.insert_library_loads(nc, mask, len(all_libraries), standard.index)
    mybir.codegen_inst_isa_subclasses(nc)
    return nc


def _prep_edges(edge_index):
    """Per-core edge arrays for dma_gather. Returns (J1, J2, per-core arrays)."""
    src = edge_index[0].astype(np.int64)
    dst = edge_index[1].astype(np.int64)
    loops = np.arange(N, dtype=np.int64)
    src = np.concatenate([src, loops])
    dst = np.concatenate([dst, loops])
    core = dst // NS
    tloc = (dst % NS) // 128
    s_loc = src % NS
    s_core = src // NS
    inA = s_loc < SPL
    rowA = s_core * SPL + s_loc                 # row in tabA (valid when inA)
    rowB = s_core * NSB + (s_loc - SPL)         # row in tabB (valid when ~inA)
    per_core = []
    j1max = j2max = 0
    for c in range(NC):
        m = core == c
        d_c, t_c = dst[m], tloc[m]
        a_c, ra_c, rb_c = inA[m], rowA[m], rowB[m]
        dl_c = (d_c % NS) - t_c * 128
        tiles = []
        for t in range(NTILE):
            mt = t_c == t
            aa = a_c[mt]
            tiles.append((ra_c[mt][aa], dl_c[mt][aa],          # segment A
                          rb_c[mt][~aa], dl_c[mt][~aa]))       # segment B
            j1max = max(j1max, -(-len(tiles[-1][0]) // 128))
            j2max = max(j2max, -(-len(tiles[-1][2]) // 128))
        per_core.append(tiles)
    J1, J2 = j1max, j2max
    NSUB = J1 + J2
    arrs = []
    for c in range(NC):
        idx16 = np.zeros((NTILE, 16, NSUB * 8), np.int16)
        dl = np.full((NTILE, 128, NSUB), 999.0, np.float32)
        for t, (rA, dA, rB, dB) in enumerate(per_core[c]):
            e = np.arange(len(rA))
            idx16[t, e % 16, e // 16] = rA.astype(np.int16)
            dl[t, e % 128, e // 128] = dA.astype(np.float32)
            e = np.arange(len(rB))
            slot = J1 * 128 + e
            idx16[t, slot % 16, J1 * 8 + e // 16] = rB.astype(np.int16)
            dl[t, slot % 128, slot // 128] = dB.astype(np.float32)
        idx16 = np.tile(idx16, (1, 8, 1))                      # replicate per 16-part group
        dlr = dl.transpose(0, 2, 1).reshape(NTILE, NSUB * 128).astype(bf)
        arrs.append((idx16, dl.astype(bf), dlr))
    return J1, J2, arrs


def kernel(x, edge_index, W0, as0, ad0, b0, g0, bt0, m0, v0,
           W1, as1, ad1, b1, g1, bt1, m1, v1,
           W2, as2, ad2, b2):
    x = np.asarray(x, np.float32)
    J1, J2, earrs = _prep_edges(np.asarray(edge_index))

    # host-side packing
    def packW(W, a_s, a_d, d_out_pad, Hh, Cc, headpad):
        # W: [d_in, d_out]; returns [128, NFB*(d_out_pad + 2H)] bf16
        d_in = W.shape[0]
        NFB = d_in // 128
        if headpad:  # L2: pad head layout (3 heads of 40 per 128-block)
            Wp = np.zeros((d_in, 256), np.float32)
            Wp[:, 0:120] = W[:, 0:120]
            Wp[:, 128:248] = W[:, 120:240]
        else:
            Wp = W.astype(np.float32)
        Wa = np.zeros((d_in, 2 * Hh), np.float32)
        for h in range(Hh):
            Wa[:, 2 * h] = W[:, h * Cc:(h + 1) * Cc] @ a_s[h]
            Wa[:, 2 * h + 1] = W[:, h * Cc:(h + 1) * Cc] @ a_d[h]
        blk = np.concatenate([Wp, Wa], axis=1)          # [d_in, d_out_pad+2H]
        blk = blk.reshape(NFB, 128, -1)
        return np.concatenate([blk[i] for i in range(NFB)], axis=1).astype(bf)

    W0 = np.asarray(W0, np.float32); W1 = np.asarray(W1, np.float32); W2 = np.asarray(W2, np.float32)
    as0 = np.asarray(as0, np.float32); ad0 = np.asarray(ad0, np.float32)
    as1 = np.asarray(as1, np.float32); ad1 = np.asarray(ad1, np.float32)
    as2 = np.asarray(as2, np.float32); ad2 = np.asarray(ad2, np.float32)
    W0p = packW(W0, as0, ad0, 512, 4, 128, False)
    W1p = packW(W1, as1, ad1, 512, 4, 128, False)
    W2p = packW(W2, as2, ad2, 256, 6, 40, True)

    def bnfold(g, bt, m, v, b):
        sc = np.asarray(g, np.float32) / np.sqrt(np.asarray(v, np.float32) + BN_EPS)
        sh = (np.asarray(b, np.float32) - np.asarray(m, np.float32)) * sc + np.asarray(bt, np.float32)
        return sc.reshape(4, 128).T.copy(), sh.reshape(4, 128).T.copy()
    sc0, sh0 = bnfold(g0, bt0, m0, v0, b0)
    sc1, sh1 = bnfold(g1, bt1, m1, v1, b1)
    b2rep = np.broadcast_to(np.asarray(b2, np.float32), (128, 40)).copy()

    iota = np.broadcast_to(np.arange(128, dtype=np.float32), (128, 128)).astype(bf)
    iotac = np.arange(128, dtype=np.float32)[:, None].astype(bf)

    xT = x.T.astype(bf)   # [128, N]
    in_maps = []
    for c in range(NC):
        xts = np.zeros((128, PADN), bf)
        xts[:, :NS] = xT[:, c * NS:(c + 1) * NS]
        idx16, dl, dlr = earrs[c]
        in_maps.append({
            "xT": xts, "W0p": W0p, "W1p": W1p, "W2p": W2p,
            "bnsc0": sc0, "bnsh0": sh0, "bnsc1": sc1, "bnsh1": sh1,
            "b2r": b2rep, "iota": iota, "iotac": iotac,
            "eidx": idx16, "edl": dl, "edlr": dlr,
        })

    global _last_in_maps, _last_j1, _last_j2
    _last_in_maps = in_maps
    _last_j1, _last_j2 = J1, J2
    nc = _build(J1, J2)
    res = run_bass_kernel_spmd(nc, in_maps, core_ids=list(range(NC)))
    out = np.concatenate([res.results[c]["out"] for c in range(NC)], axis=0)
    return out.astype(np.float32)


# revision 18
# speedup vs baseline: 1.3604x; 1.2170x over previous
"""3-layer GAT (arXiv-style) on 8 Trainium2 NeuronCores via Bass.

Sharding: dst-node sharding (6250 nodes/core). Node phase computes each
core's h-table slice [h | a_src-dot] rows, split at local row 3072 into two
tables; two AllGathers replicate them (each <32768 rows so dma_gather's
int16 indices reach every row). Edge phase gathers per-edge source rows
with gpsimd dma_gather (<=1024 indices/call, per-tile exact widths),
handles self-loops via a direct DMA + identity matmul, builds 0/1
selection matrices from dst-locals vs iota, and does segment-softmax +
feature aggregation as PSUM-accumulated matmuls, pipelined per segment.
Output: log_softmax logits.
"""
import numpy as np
import ml_dtypes

import concourse.bass as bass
import concourse.mybir as mybir
import concourse.tile as tile
from concourse.bass_utils import run_bass_kernel_spmd
from concourse.library_config import all_libraries, standard
import bass_rust

# ---- problem constants (hardcoded per harness contract) ----
N = 50000
E = 800000
F_IN = 128
NEG = 0.2
BN_EPS = 1e-5
NC = 8
NS = N // NC            # 6250 nodes per core
NTILE = (NS + 127) // 128   # 49 dst tiles per core
PADN = 13 * 512         # node-phase padded slice rows (6656)
NCHUNK = 13             # pout row chunks (512 rows each) for overlap
SPL = 3072              # local-row split: tblA rows [0,3072), tblB [3072,6250)
NSB = NS - SPL          # 3178 rows per core in tblB
ROW1 = 640              # layer0/1 table row: h(512)+as(4)+pad (1280B, %256)
ROW2 = 256              # layer2 table row: h(240)+as(6)+ad(6)+pad (512B, %256)
H12, C12 = 4, 128
H2, C2 = 6, 40
MAXW = 8                # dma_gather call width cap (1024-descriptor ring)
AF = mybir.ActivationFunctionType
ALU = mybir.AluOpType
dt = mybir.dt
F32, BF16, I16 = dt.float32, dt.bfloat16, dt.int16
bf = ml_dtypes.bfloat16


def _hoist_waits(nc, max_keep=1):
    n = 0
    for f in nc.m.functions:
        for bb in f.blocks:
            out, changed = [], False
            for ins in bb.instructions:
                si = getattr(ins, "sync_info", None)
                if si is not None and si.on_wait:
                    keep = 0 if (isinstance(ins, mybir.InstDMAGatherAnt)
                                 or (isinstance(ins, mybir.InstDMACopy)
                                     and getattr(ins, "queue", None) == "qPoolDynamic")) else max_keep
                    waits = list(si.on_wait)
                    if len(waits) > keep:
                        cut = len(waits) - keep
                        for w in waits[:cut]:
                            out.append(mybir.InstEventSemaphore(
                                name=f"I-hw-{n}", engine=ins.engine, ins=[], outs=[],
                                sync_info=mybir.SyncInfo(on_wait=[w], on_update=[])))
                            n += 1
                        si.on_wait = waits[cut:]
                        changed = True
                out.append(ins)
            if changed:
                bb.instructions = out
    return n


def _call_widths(jseg):
    """Split a segment of jseg 128-idx columns into balanced calls of <=MAXW."""
    if jseg == 0:
        return []
    ncalls = -(-jseg // MAXW)
    base = jseg // ncalls
    rem = jseg - base * ncalls
    return [base + (1 if i < rem else 0) for i in range(ncalls)]


def _leaky_exp_w(nc, sb, H, nsub, sE, tag):
    """w = exp(leaky_relu(sE)); returns bf16 wb tile [128, nsub*H]."""
    m2 = sb.tile([128, nsub * H], F32, tag=tag + "m")
    nc.vector.tensor_scalar(out=m2[:], in0=sE[:], scalar1=0.0, scalar2=NEG,
                            op0=ALU.min, op1=ALU.mult)
    lr = sb.tile([128, nsub * H], F32, tag=tag + "l")
    nc.vector.scalar_tensor_tensor(out=lr[:], in0=sE[:], scalar=0.0, in1=m2[:],
                                   op0=ALU.max, op1=ALU.add)
    w_ = sb.tile([128, nsub * H], F32, tag=tag + "w")
    nc.scalar.activation(out=w_[:], in_=lr[:], func=AF.Exp)
    wb = sb.tile([128, nsub * H], BF16, tag=tag + "b")
    nc.vector.tensor_copy(out=wb[:], in_=w_[:])
    return wb


def _edge_phase(nc, sb, ps, lay, meta, tabA, tabB, tblA, tblB, idx16_all, dl_all,
                dlr_flat, adsl, io, ioc, ident, pout_next, out_final, b2r, wregs):
    """One layer's edge phase: 49 dst tiles, per-segment pipelined."""
    ROW = ROW1 if lay < 2 else ROW2
    H = H12 if lay < 2 else H2
    C = C12 if lay < 2 else C2
    HC = H * C
    J1s, J2s, off16, offdl, offdlr, J1M, J2M = meta
    for t in range(NTILE):
        nreal = min(128, NS - t * 128)
        J1, J2 = J1s[t], J2s[t]
        NSUB = J1 + J2
        o16, odl, odlr = off16[t], offdl[t], offdlr[t]

        adt = sb.tile([128, 8], BF16, tag="eadt")
        nc.sync.dma_start(out=adt[:, :H], in_=adsl[t * 128:(t + 1) * 128, :H])
        adtf = sb.tile([128, 8], F32, tag="eadtf")
        nc.vector.tensor_copy(out=adtf[:, :H], in_=adt[:, :H])

        # ---- self-loop column: direct DMA from the local table slice ----
        Gs = sb.tile([128, ROW], BF16, tag="eGs")
        if t * 128 < SPL:
            nc.sync.dma_start(out=Gs[:], in_=tblA[t * 128:t * 128 + 128, :])
        else:
            r0 = t * 128 - SPL
            k = min(128, NSB - r0)
            nc.sync.dma_start(out=Gs[:k, :], in_=tblB[r0:r0 + k, :])
        asGs = sb.tile([128, 8], F32, tag="easGs")
        nc.vector.tensor_copy(out=asGs[:, :H], in_=Gs[:, HC:HC + H])
        sEs = sb.tile([128, 8], F32, tag="esEs")
        nc.vector.tensor_add(out=sEs[:, :H], in0=asGs[:, :H], in1=adtf[:, :H])
        wbs = _leaky_exp_w(nc, sb, H, 1, sEs[:, :H], "ws")
        Gws = sb.tile([128, HC], BF16, tag="eGws")
        nc.vector.tensor_tensor(
            out=Gws[:].rearrange("p (h c) -> p h c", h=H),
            in0=Gs[:, 0:HC].rearrange("p (h c) -> p h c", h=H),
            in1=wbs[:].rearrange("p (h o) -> p h o", o=1).to_broadcast([128, H, C]),
            op=ALU.mult)
        pnum = ps.tile([128, HC], F32, space="PSUM", tag="pnum")
        pden = ps.tile([128, H], F32, space="PSUM", tag="pden")
        nc.tensor.matmul(out=pnum[:], lhsT=ident[:], rhs=Gws[:],
                         start=True, stop=False)
        nc.tensor.matmul(out=pden[:], lhsT=ident[:], rhs=wbs[:],
                         start=True, stop=False)

        # ---- gathered segments A (tabA) and B (tabB) ----
        dlr = sb.tile([128, (J1M + J2M) * 128], BF16, tag="edlr")
        nc.sync.dma_start(
            out=dlr[:, :NSUB * 128],
            in_=dlr_flat[0:1, odlr:odlr + NSUB * 128].to_broadcast([128, NSUB * 128]))

        segs = []
        col = 0
        for si, (tab, jseg, JM) in enumerate(((tabA, J1, J1M), (tabB, J2, J2M))):
            G = sb.tile([128, JM * ROW], BF16, tag=f"eG{si}")
            c0 = col
            for w in _call_widths(jseg):
                nc.gpsimd.dma_gather(
                    G[:, (col - c0) * ROW:(col - c0 + w) * ROW]
                        .rearrange("p (j r) -> p j r", r=ROW),
                    tab[:], idx16_all[:, o16 + col * 8:o16 + (col + w) * 8],
                    w * 128, wregs[w], ROW)
                col += w
            segs.append((G, jseg, c0))

        for si, (G, jseg, c0) in enumerate(segs):
            if jseg == 0:
                continue
            S = sb.tile([128, (J1M if si == 0 else J2M) * 128], BF16, tag=f"eS{si}")
            nc.vector.tensor_tensor(
                out=S[:, :jseg * 128].rearrange("p (n f) -> p n f", f=128),
                in0=dl_all[:, odl + c0:odl + c0 + jseg]
                    .rearrange("p (n o) -> p n o", o=1).to_broadcast([128, jseg, 128]),
                in1=io[:].rearrange("p (o f) -> p o f", o=1).to_broadcast([128, jseg, 128]),
                op=ALU.is_equal)
            ST = sb.tile([128, (J1M if si == 0 else J2M) * 128], BF16, tag=f"eST{si}")
            nc.vector.tensor_tensor(
                out=ST[:, :jseg * 128], in0=dlr[:, c0 * 128:(c0 + jseg) * 128],
                in1=ioc[:].to_broadcast([128, jseg * 128]), op=ALU.is_equal)

            pad_ = ps.tile([128, (J1M if si == 0 else J2M) * H], F32,
                           space="PSUM", tag=f"pad{si}")
            for j in range(jseg):
                nc.tensor.matmul(out=pad_[:, H * j:H * (j + 1)],
                                 lhsT=ST[:, j * 128:(j + 1) * 128],
                                 rhs=adt[:, :H], start=True, stop=True)

            asE = sb.tile([128, (J1M if si == 0 else J2M) * H], F32, tag=f"easE{si}")
            nc.vector.tensor_copy(
                out=asE[:, :jseg * H].rearrange("p (n f) -> p n f", f=H),
                in_=G[:, :jseg * ROW].rearrange("p (n e) -> p n e", e=ROW)[:, :, HC:HC + H])
            sE = sb.tile([128, (J1M if si == 0 else J2M) * H], F32, tag=f"esE{si}")
            nc.vector.tensor_add(out=sE[:, :jseg * H], in0=asE[:, :jseg * H],
                                 in1=pad_[:, :jseg * H])
            wb = _leaky_exp_w(nc, sb, H, jseg, sE[:, :jseg * H], f"w{si}")

            Gw = sb.tile([128, (J1M if si == 0 else J2M) * HC], BF16, tag=f"eGw{si}")
            nc.vector.tensor_tensor(
                out=Gw[:, :jseg * HC].rearrange("p (n h c) -> p n h c", h=H, c=C),
                in0=G[:, :jseg * ROW].rearrange("p (n e) -> p n e", e=ROW)[:, :, 0:HC]
                     .rearrange("p n (h c) -> p n h c", h=H),
                in1=wb[:, :jseg * H].rearrange("p (n h o) -> p n h o", h=H, o=1)
                     .to_broadcast([128, jseg, H, C]),
                op=ALU.mult)

            last = (si == (1 if J2 > 0 else 0))
            for j in range(jseg):
                nc.tensor.matmul(out=pnum[:], lhsT=S[:, j * 128:(j + 1) * 128],
                                 rhs=Gw[:, j * HC:(j + 1) * HC],
                                 start=False, stop=last and (j == jseg - 1))
                nc.tensor.matmul(out=pden[:], lhsT=S[:, j * 128:(j + 1) * 128],
                                 rhs=wb[:, j * H:(j + 1) * H],
                                 start=False, stop=last and (j == jseg - 1))

        rden = sb.tile([128, H], F32, tag="erden")
        nc.vector.reciprocal(out=rden[:], in_=pden[:])

        if lay < 2:
            ot = sb.tile([128, 512], BF16, tag="eot")
            for hh in range(H):
                nc.scalar.activation(out=ot[:, hh * C:(hh + 1) * C],
                                     in_=pnum[:, hh * C:(hh + 1) * C],
                                     func=AF.Identity, scale=rden[:, hh:hh + 1])
            ck, cr = (t * 128) // 512, (t * 128) % 512
            nc.sync.dma_start(out=pout_next[ck][cr:cr + nreal, :], in_=ot[:nreal, :])
        else:
            tmp = sb.tile([128, 240], F32, tag="etmp")
            for hh in range(H):
                nc.scalar.activation(out=tmp[:, hh * C:(hh + 1) * C],
                                     in_=pnum[:, hh * C:(hh + 1) * C],
                                     func=AF.Identity, scale=rden[:, hh:hh + 1])
            acc = sb.tile([128, 40], F32, tag="eacc")
            nc.vector.tensor_add(out=acc[:], in0=tmp[:, 0:40], in1=tmp[:, 40:80])
            for hh in range(2, 6):
                nc.vector.tensor_add(out=acc[:], in0=acc[:], in1=tmp[:, hh * 40:(hh + 1) * 40])
            z = sb.tile([128, 40], F32, tag="ez")
            nc.vector.tensor_scalar_mul(out=z[:], in0=acc[:], scalar1=1.0 / 6.0)
            nc.vector.tensor_add(out=z[:], in0=z[:], in1=b2r[:])
            mx = sb.tile([128, 1], F32, tag="emx")
            nc.vector.reduce_max(out=mx[:], in_=z[:], axis=mybir.AxisListType.X)
            zs = sb.tile([128, 40], F32, tag="ezs")
            nc.vector.tensor_scalar(out=zs[:], in0=z[:], scalar1=mx[:, 0:1], scalar2=None,
                                    op0=ALU.subtract)
            ex = sb.tile([128, 40], F32, tag="eex")
            nc.scalar.activation(out=ex[:], in_=zs[:], func=AF.Exp)
            sm = sb.tile([128, 1], F32, tag="esm")
            nc.vector.reduce_sum(out=sm[:], in_=ex[:], axis=mybir.AxisListType.X)
            ln = sb.tile([128, 1], F32, tag="eln")
            nc.scalar.activation(out=ln[:], in_=sm[:], func=AF.Ln)
            lsm = sb.tile([128, 40], F32, tag="elsm")
            nc.vector.tensor_scalar(out=lsm[:], in0=zs[:], scalar1=ln[:, 0:1], scalar2=None,
                                    op0=ALU.subtract)
            nc.sync.dma_start(out=out_final[t * 128:t * 128 + nreal, :], in_=lsm[:nreal, :])


def _node_phase(nc, sb, ps, lay, xin, pout, wt, blk, bn_sc, bn_sh, tblA, tblB, adsl):
    """Node phase: compute table slice rows [h | as] for this core's nodes."""
    if lay == 0:
        d_in, d_out, H, ROW, HC = 128, 512, 4, ROW1, 512
    elif lay == 1:
        d_in, d_out, H, ROW, HC = 512, 512, 4, ROW1, 512
    else:
        d_in, d_out, H, ROW, HC = 512, 256, 6, ROW2, 240
    NFB = d_in // 128
    for it in range(13):
        n0 = it * 512
        aT = []
        for fb in range(NFB):
            if lay == 0:
                aT.append(xin[:, n0:n0 + 512])
                continue
            a = sb.tile([128, 512], BF16, tag=f"naT{fb}")
            zb = sb.tile([128, 512], BF16, tag="nzb")
            nc.sync.dma_start(out=zb[:],
                              in_=pout[it][:, fb * 128:(fb + 1) * 128],
                              transpose=True)
            z = sb.tile([128, 512], F32, tag="nzf")
            nc.scalar.activation(out=z[:], in_=zb[:], func=AF.Identity,
                                 bias=bn_sh[:, fb:fb + 1], scale=bn_sc[:, fb:fb + 1])
            mm = sb.tile([128, 512], F32, tag="nmm")
            nc.vector.tensor_scalar_min(out=mm[:], in0=z[:], scalar1=0.0)
            ee = sb.tile([128, 512], F32, tag="nee")
            nc.scalar.activation(out=ee[:], in_=mm[:], func=AF.Exp)
            rr = sb.tile([128, 512], F32, tag="nrr")
            nc.vector.scalar_tensor_tensor(out=rr[:], in0=z[:], scalar=0.0, in1=ee[:],
                                           op0=ALU.max, op1=ALU.add)
            nc.vector.tensor_scalar(out=a[:], in0=rr[:], scalar1=-1.0, scalar2=None,
                                    op0=ALU.add)
            aT.append(a[:])
        for nb in range(4):
            ph = ps.tile([128, d_out], F32, space="PSUM", tag="nph")
            pa = ps.tile([128, 2 * H], F32, space="PSUM", tag="npa")
            for fb in range(NFB):
                lhsT = aT[fb][:, nb * 128:(nb + 1) * 128]
                nc.tensor.matmul(out=ph[:], lhsT=lhsT,
                                 rhs=wt[:, fb * blk:fb * blk + d_out],
                                 start=(fb == 0), stop=(fb == NFB - 1))
                nc.tensor.matmul(out=pa[:], lhsT=lhsT,
                                 rhs=wt[:, fb * blk + d_out:(fb + 1) * blk],
                                 start=(fb == 0), stop=(fb == NFB - 1))
            row0 = n0 + nb * 128
            nreal = min(128, max(0, NS - row0))
            if nreal == 0:
                continue
            used = HC + H if lay < 2 else ROW2
            tb = sb.tile([128, used], BF16, tag="ntb")
            if lay < 2:
                nc.vector.tensor_copy(out=tb[:, 0:512], in_=ph[:])
            else:
                nc.vector.tensor_copy(out=tb[:, 0:120], in_=ph[:, 0:120])
                nc.vector.tensor_copy(out=tb[:, 120:240], in_=ph[:, 128:248])
            nc.vector.tensor_copy(out=tb[:, HC:HC + H].rearrange("p (n o) -> p n o", o=1),
                                  in_=pa[:].rearrange("p (n s) -> p n s", s=2)[:, :, 0:1])
            if lay == 2:
                nc.vector.tensor_copy(
                    out=tb[:, HC + H:HC + 2 * H].rearrange("p (n o) -> p n o", o=1),
                    in_=pa[:].rearrange("p (n s) -> p n s", s=2)[:, :, 1:2])
            if row0 < SPL:
                nc.sync.dma_start(out=tblA[row0:row0 + nreal, 0:used], in_=tb[:nreal, :])
            else:
                nc.sync.dma_start(out=tblB[row0 - SPL:row0 - SPL + nreal, 0:used],
                                  in_=tb[:nreal, :])
            ab = sb.tile([128, 8], BF16, tag="nab")
            nc.vector.tensor_copy(out=ab[:, :H].rearrange("p (n o) -> p n o", o=1),
                                  in_=pa[:].rearrange("p (n s) -> p n s", s=2)[:, :, 1:2])
            nc.sync.dma_start(out=adsl[row0:row0 + nreal, :H], in_=ab[:nreal, :H])


def _build(meta):
    J1s, J2s, off16, offdl, offdlr, J1M, J2M = meta
    TOT16 = off16[-1]
    TOTDL = offdl[-1]
    TOTDLR = offdlr[-1]
    nc = bass.Bass()
    # ---- inputs (per-core) ----
    xT = nc.declare_dram_parameter("xT", [128, PADN], BF16, isOutput=False)
    W0 = nc.declare_dram_parameter("W0p", [128, 512 + 8], BF16, isOutput=False)
    W1 = nc.declare_dram_parameter("W1p", [128, 4 * (512 + 8)], BF16, isOutput=False)
    W2 = nc.declare_dram_parameter("W2p", [128, 4 * (256 + 12)], BF16, isOutput=False)
    bnsc0 = nc.declare_dram_parameter("bnsc0", [128, 4], F32, isOutput=False)
    bnsh0 = nc.declare_dram_parameter("bnsh0", [128, 4], F32, isOutput=False)
    bnsc1 = nc.declare_dram_parameter("bnsc1", [128, 4], F32, isOutput=False)
    bnsh1 = nc.declare_dram_parameter("bnsh1", [128, 4], F32, isOutput=False)
    b2r = nc.declare_dram_parameter("b2r", [128, 40], F32, isOutput=False)
    iota = nc.declare_dram_parameter("iota", [128, 128], BF16, isOutput=False)
    iotac = nc.declare_dram_parameter("iotac", [128, 1], BF16, isOutput=False)
    eidx = nc.declare_dram_parameter("eidx", [128, TOT16], I16, isOutput=False)
    edl = nc.declare_dram_parameter("edl", [128, TOTDL], BF16, isOutput=False)
    edlr = nc.declare_dram_parameter("edlr", [1, TOTDLR], BF16, isOutput=False)
    out = nc.declare_dram_parameter("out", [NS, 40], F32, isOutput=True)
    # ---- internal ----
    tblA0 = nc.dram_tensor("tblA0", [SPL, ROW1], BF16)
    tblB0 = nc.dram_tensor("tblB0", [NSB, ROW1], BF16)
    tblA1 = nc.dram_tensor("tblA1", [SPL, ROW1], BF16)
    tblB1 = nc.dram_tensor("tblB1", [NSB, ROW1], BF16)
    tblA2 = nc.dram_tensor("tblA2", [SPL, ROW2], BF16)
    tblB2 = nc.dram_tensor("tblB2", [NSB, ROW2], BF16)
    tabA0 = nc.dram_tensor("tabA0", [NC * SPL, ROW1], BF16, addr_space="Shared")
    tabB0 = nc.dram_tensor("tabB0", [NC * NSB, ROW1], BF16, addr_space="Shared")
    tabA1 = nc.dram_tensor("tabA1", [NC * SPL, ROW1], BF16, addr_space="Shared")
    tabB1 = nc.dram_tensor("tabB1", [NC * NSB, ROW1], BF16, addr_space="Shared")
    tabA2 = nc.dram_tensor("tabA2", [NC * SPL, ROW2], BF16, addr_space="Shared")
    tabB2 = nc.dram_tensor("tabB2", [NC * NSB, ROW2], BF16, addr_space="Shared")
    pout0 = [nc.dram_tensor(f"pout0_{k}", [512, 512], BF16) for k in range(NCHUNK)]
    pout1 = [nc.dram_tensor(f"pout1_{k}", [512, 512], BF16) for k in range(NCHUNK)]
    adsl0 = nc.dram_tensor("adsl0", [NTILE * 128, 8], BF16)
    adsl1 = nc.dram_tensor("adsl1", [NTILE * 128, 8], BF16)
    adsl2 = nc.dram_tensor("adsl2", [NTILE * 128, 8], BF16)

    rg = [list(range(NC))]
    with tile.TileContext(nc) as tc:
        with tc.tile_pool(name="cst", bufs=1) as cst, \
             tc.tile_pool(name="sb", bufs=2) as sb, \
             tc.tile_pool(name="ps", bufs=1, space="PSUM") as ps:
            io = cst.tile([128, 128], BF16)
            nc.sync.dma_start(out=io[:], in_=iota[:])
            ioc = cst.tile([128, 1], BF16)
            nc.sync.dma_start(out=ioc[:], in_=iotac[:])
            ident = cst.tile([128, 128], BF16)
            nc.vector.tensor_tensor(out=ident[:], in0=ioc[:].to_broadcast([128, 128]),
                                    in1=io[:], op=ALU.is_equal)
            b2 = cst.tile([128, 40], F32)
            nc.sync.dma_start(out=b2[:], in_=b2r[:])
            xt = cst.tile([128, PADN], BF16)
            nc.sync.dma_start(out=xt[:], in_=xT[:])
            w0 = cst.tile([128, 520], BF16)
            nc.sync.dma_start(out=w0[:], in_=W0[:])
            w1 = cst.tile([128, 4 * 520], BF16)
            nc.sync.dma_start(out=w1[:], in_=W1[:])
            w2 = cst.tile([128, 4 * 268], BF16)
            nc.sync.dma_start(out=w2[:], in_=W2[:])
            sc0 = cst.tile([128, 4], F32); nc.sync.dma_start(out=sc0[:], in_=bnsc0[:])
            sh0 = cst.tile([128, 4], F32); nc.sync.dma_start(out=sh0[:], in_=bnsh0[:])
            sc1 = cst.tile([128, 4], F32); nc.sync.dma_start(out=sc1[:], in_=bnsc1[:])
            sh1 = cst.tile([128, 4], F32); nc.sync.dma_start(out=sh1[:], in_=bnsh1[:])
            idx16_all = cst.tile([128, TOT16], I16)
            nc.sync.dma_start(out=idx16_all[:], in_=eidx[:])
            dl_all = cst.tile([128, TOTDL], BF16)
            nc.sync.dma_start(out=dl_all[:], in_=edl[:])
            zz = cst.tile([128, 512], BF16)
            nc.gpsimd.memset(zz[:], 0.0)
            wregs = {w: nc.gpsimd.to_reg(w * 128)
                     for w in set(sum((_call_widths(j) for j in J1s + J2s), []))}
            # zero pad tail of last pout chunk (rows 6250.. of global layout)
            for pout in (pout0, pout1):
                r = NS - 12 * 512
                while r < 512:
                    k = min(128, 512 - r)
                    nc.sync.dma_start(out=pout[NCHUNK - 1][r:r + k, :], in_=zz[:k, :])
                    r += k
            for adsl in (adsl0, adsl1, adsl2):
                nc.sync.dma_start(out=adsl[NS:NTILE * 128, :], in_=zz[:NTILE * 128 - NS, :8])

            # Layer 0
            _node_phase(nc, sb, ps, 0, xt[:], None, w0[:], 520, None, None,
                        tblA0, tblB0, adsl0)
            nc.gpsimd.collective_compute("AllGather", ALU.bypass, replica_groups=rg,
                                         ins=[tblA0[:]], outs=[tabA0[:]])
            nc.gpsimd.collective_compute("AllGather", ALU.bypass, replica_groups=rg,
                                         ins=[tblB0[:]], outs=[tabB0[:]])
            _edge_phase(nc, sb, ps, 0, meta, tabA0, tabB0, tblA0, tblB0, idx16_all,
                        dl_all, edlr, adsl0, io, ioc, ident, pout0, None, None, wregs)
            # Layer 1
            _node_phase(nc, sb, ps, 1, None, pout0, w1[:], 520, sc0[:], sh0[:],
                        tblA1, tblB1, adsl1)
            nc.gpsimd.collective_compute("AllGather", ALU.bypass, replica_groups=rg,
                                         ins=[tblA1[:]], outs=[tabA1[:]])
            nc.gpsimd.collective_compute("AllGather", ALU.bypass, replica_groups=rg,
                                         ins=[tblB1[:]], outs=[tabB1[:]])
            _edge_phase(nc, sb, ps, 1, meta, tabA1, tabB1, tblA1, tblB1, idx16_all,
                        dl_all, edlr, adsl1, io, ioc, ident, pout1, None, None, wregs)
            # Layer 2
            _node_phase(nc, sb, ps, 2, None, pout1, w2[:], 268, sc1[:], sh1[:],
                        tblA2, tblB2, adsl2)
            nc.gpsimd.collective_compute("AllGather", ALU.bypass, replica_groups=rg,
                                         ins=[tblA2[:]], outs=[tabA2[:]])
            nc.gpsimd.collective_compute("AllGather", ALU.bypass, replica_groups=rg,
                                         ins=[tblB2[:]], outs=[tabB2[:]])
            _edge_phase(nc, sb, ps, 2, meta, tabA2, tabB2, tblA2, tblB2, idx16_all,
                        dl_all, edlr, adsl2, io, ioc, ident, None, out, b2[:], wregs)
    _hoist_waits(nc)
    mask = {}
    for lib in all_libraries:
        for ty in lib.instructions:
            mask[ty] = mask.get(ty, 0) | (1 << lib.index)
    bass_rust.insert_library_loads(nc, mask, len(all_libraries), standard.index)
    mybir.codegen_inst_isa_subclasses(nc)
    return nc


def _prep_edges(edge_index):
    """Per-core edge arrays for dma_gather (self-loops excluded).

    Returns (meta, per-core (idx16, dl, dlr) arrays) where meta =
    (J1s, J2s, off16, offdl, offdlr, J1M, J2M); offsets have NTILE+1
    entries (element counts into the flat arrays).
    """
    src = edge_index[0].astype(np.int64)
    dst = edge_index[1].astype(np.int64)
    core = dst // NS
    tloc = (dst % NS) // 128
    s_loc = src % NS
    s_core = src // NS
    inA = s_loc < SPL
    rowA = s_core * SPL + s_loc
    rowB = s_core * NSB + (s_loc - SPL)
    percore = []
    for c in range(NC):
        m = core == c
        d_c, t_c = dst[m], tloc[m]
        a_c, ra_c, rb_c = inA[m], rowA[m], rowB[m]
        dl_c = (d_c % NS) - t_c * 128
        tiles = []
        for t in range(NTILE):
            mt = t_c == t
            aa = a_c[mt]
            tiles.append((ra_c[mt][aa], dl_c[mt][aa],
                          rb_c[mt][~aa], dl_c[mt][~aa]))
        percore.append(tiles)
    J1s, J2s = [], []
    for t in range(NTILE):
        J1s.append(max(-(-len(percore[c][t][0]) // 128) for c in range(NC)))
        J2s.append(max(-(-len(percore[c][t][2]) // 128) for c in range(NC)))
    J1M, J2M = max(J1s), max(J2s)
    off16 = [0]; offdl = [0]; offdlr = [0]
    for t in range(NTILE):
        ns = J1s[t] + J2s[t]
        off16.append(off16[-1] + ns * 8)
        offdl.append(offdl[-1] + ns)
        offdlr.append(offdlr[-1] + ns * 128)
    meta = (J1s, J2s, off16, offdl, offdlr, J1M, J2M)
    arrs = []
    for c in range(NC):
        idx16 = np.zeros((16, off16[-1]), np.int16)
        dlf = np.full((128, offdl[-1]), 999.0, np.float32)
        dlrf = np.full((1, offdlr[-1]), 999.0, np.float32)
        for t in range(NTILE):
            rA, dA, rB, dB = percore[c][t]
            for (rr, dd, joff) in ((rA, dA, 0), (rB, dB, J1s[t])):
                e = np.arange(len(rr))
                idx16[e % 16, off16[t] + joff * 8 + e // 16] = rr.astype(np.int16)
                dlf[e % 128, offdl[t] + joff + e // 128] = dd
                dlrf[0, offdlr[t] + joff * 128 + (e // 128) * 128 + e % 128] = dd
        arrs.append((np.tile(idx16, (8, 1)), dlf.astype(bf), dlrf.astype(bf)))
    return meta, arrs


def kernel(x, edge_index, W0, as0, ad0, b0, g0, bt0, m0, v0,
           W1, as1, ad1, b1, g1, bt1, m1, v1,
           W2, as2, ad2, b2):
    x = np.asarray(x, np.float32)
    meta, earrs = _prep_edges(np.asarray(edge_index))

    # host-side packing
    def packW(W, a_s, a_d, d_out_pad, Hh, Cc, headpad):
        # W: [d_in, d_out]; returns [128, NFB*(d_out_pad + 2H)] bf16
        d_in = W.shape[0]
        NFB = d_in // 128
        if headpad:  # L2: pad head layout (3 heads of 40 per 128-block)
            Wp = np.zeros((d_in, 256), np.float32)
            Wp[:, 0:120] = W[:, 0:120]
            Wp[:, 128:248] = W[:, 120:240]
        else:
            Wp = W.astype(np.float32)
        Wa = np.zeros((d_in, 2 * Hh), np.float32)
        for h in range(Hh):
            Wa[:, 2 * h] = W[:, h * Cc:(h + 1) * Cc] @ a_s[h]
            Wa[:, 2 * h + 1] = W[:, h * Cc:(h + 1) * Cc] @ a_d[h]
        blk = np.concatenate([Wp, Wa], axis=1)          # [d_in, d_out_pad+2H]
        blk = blk.reshape(NFB, 128, -1)
        return np.concatenate([blk[i] for i in range(NFB)], axis=1).astype(bf)

    W0 = np.asarray(W0, np.float32); W1 = np.asarray(W1, np.float32); W2 = np.asarray(W2, np.float32)
    as0 = np.asarray(as0, np.float32); ad0 = np.asarray(ad0, np.float32)
    as1 = np.asarray(as1, np.float32); ad1 = np.asarray(ad1, np.float32)
    as2 = np.asarray(as2, np.float32); ad2 = np.asarray(ad2, np.float32)
    W0p = packW(W0, as0, ad0, 512, 4, 128, False)
    W1p = packW(W1, as1, ad1, 512, 4, 128, False)
    W2p = packW(W2, as2, ad2, 256, 6, 40, True)

    def bnfold(g, bt, m, v, b):
        sc = np.asarray(g, np.float32) / np.sqrt(np.asarray(v, np.float32) + BN_EPS)
        sh = (np.asarray(b, np.float32) - np.asarray(m, np.float32)) * sc + np.asarray(bt, np.float32)
        return sc.reshape(4, 128).T.copy(), sh.reshape(4, 128).T.copy()
    sc0, sh0 = bnfold(g0, bt0, m0, v0, b0)
    sc1, sh1 = bnfold(g1, bt1, m1, v1, b1)
    b2rep = np.broadcast_to(np.asarray(b2, np.float32), (128, 40)).copy()

    iota = np.broadcast_to(np.arange(128, dtype=np.float32), (128, 128)).astype(bf)
    iotac = np.arange(128, dtype=np.float32)[:, None].astype(bf)

    xT = x.T.astype(bf)   # [128, N]
    in_maps = []
    for c in range(NC):
        xts = np.zeros((128, PADN), bf)
        xts[:, :NS] = xT[:, c * NS:(c + 1) * NS]
        idx16, dlf, dlrf = earrs[c]
        in_maps.append({
            "xT": xts, "W0p": W0p, "W1p": W1p, "W2p": W2p,
            "bnsc0": sc0, "bnsh0": sh0, "bnsc1": sc1, "bnsh1": sh1,
            "b2r": b2rep, "iota": iota, "iotac": iotac,
            "eidx": idx16, "edl": dlf, "edlr": dlrf,
        })

    global _last_in_maps, _last_meta
    _last_in_maps = in_maps
    _last_meta = meta
    nc = _build(meta)
    res = run_bass_kernel_spmd(nc, in_maps, core_ids=list(range(NC)))
    out = np.concatenate([res.results[c]["out"] for c in range(NC)], axis=0)
    return out.astype(np.float32)


# revision 19
# speedup vs baseline: 1.5483x; 1.1382x over previous
"""3-layer GAT (arXiv-style) on 8 Trainium2 NeuronCores via Bass.

Sharding: dst-node sharding (6250 nodes/core). Node phase computes each
core's h-table slice [h | a_src-dot] rows, split at local row 2688 into two
tables; two AllGathers replicate them (each <32768 rows so dma_gather's
int16 indices reach every row). Edge phase gathers per-edge source rows
with gpsimd dma_gather (<=1024 indices/call, per-tile exact widths),
handles self-loops via a direct DMA + identity matmul, builds 0/1
selection matrices from dst-locals vs iota, and does segment-softmax +
feature aggregation as PSUM-accumulated matmuls, pipelined per segment.
The next layer's node chunks and the first AllGather are interleaved into
the edge tile loop so collectives overlap gathers. Output: log_softmax.
"""
import numpy as np
import ml_dtypes

import concourse.bass as bass
import concourse.mybir as mybir
import concourse.tile as tile
from concourse.bass_utils import run_bass_kernel_spmd
from concourse.library_config import all_libraries, standard
import bass_rust

# ---- problem constants (hardcoded per harness contract) ----
N = 50000
E = 800000
F_IN = 128
NEG = 0.2
BN_EPS = 1e-5
NC = 8
NS = N // NC            # 6250 nodes per core
NTILE = (NS + 127) // 128   # 49 dst tiles per core
PADN = 13 * 512         # node-phase padded slice rows (6656)
NCHUNK = 13             # pout row chunks (512 rows each) for overlap
SPL = 2688              # local-row split: tblA rows [0,2688), tblB [2688,6250)
NSB = NS - SPL          # 3562 rows per core in tblB
ROW1 = 640              # layer0/1 table row: h(512)+as(4)+pad (1280B, %256)
ROW2 = 256              # layer2 table row: h(240)+as(6)+ad(6)+pad (512B, %256)
H12, C12 = 4, 128
H2, C2 = 6, 40
MAXW = 8                # dma_gather call width cap (1024-descriptor ring)
AF = mybir.ActivationFunctionType
ALU = mybir.AluOpType
dt = mybir.dt
F32, BF16, I16 = dt.float32, dt.bfloat16, dt.int16
bf = ml_dtypes.bfloat16


def _hoist_waits(nc, max_keep=1):
    n = 0
    for f in nc.m.functions:
        for bb in f.blocks:
            out, changed = [], False
            for ins in bb.instructions:
                si = getattr(ins, "sync_info", None)
                if si is not None and si.on_wait:
                    keep = 0 if (isinstance(ins, mybir.InstDMAGatherAnt)
                                 or (isinstance(ins, mybir.InstDMACopy)
                                     and getattr(ins, "queue", None) == "qPoolDynamic")) else max_keep
                    waits = list(si.on_wait)
                    if len(waits) > keep:
                        cut = len(waits) - keep
                        for w in waits[:cut]:
                            out.append(mybir.InstEventSemaphore(
                                name=f"I-hw-{n}", engine=ins.engine, ins=[], outs=[],
                                sync_info=mybir.SyncInfo(on_wait=[w], on_update=[])))
                            n += 1
                        si.on_wait = waits[cut:]
                        changed = True
                out.append(ins)
            if changed:
                bb.instructions = out
    return n


def _call_widths(jseg):
    """Split a segment of jseg 128-idx columns into balanced calls of <=MAXW."""
    if jseg == 0:
        return []
    ncalls = -(-jseg // MAXW)
    base = jseg // ncalls
    rem = jseg - base * ncalls
    return [base + (1 if i < rem else 0) for i in range(ncalls)]


def _leaky_exp_w(nc, sb, H, nsub, sE, tag):
    """wb = exp(leaky_relu(sE)) in bf16; sE is a f32 AP [128, nsub*H]."""
    m2 = sb.tile([128, nsub * H], F32, tag=tag + "m")
    nc.vector.tensor_scalar(out=m2[:], in0=sE, scalar1=0.0, scalar2=NEG,
                            op0=ALU.min, op1=ALU.mult)
    lr = sb.tile([128, nsub * H], F32, tag=tag + "l")
    nc.vector.scalar_tensor_tensor(out=lr[:], in0=sE, scalar=0.0, in1=m2[:],
                                   op0=ALU.max, op1=ALU.add)
    wb = sb.tile([128, nsub * H], BF16, tag=tag + "b")
    nc.scalar.activation(out=wb[:], in_=lr[:], func=AF.Exp)
    return wb


def _edge_phase(nc, sb, ps, lay, meta, tabA, tabB, tblA, tblB, idx16_all, dl_all,
                dlr_flat, adsl, io, ioc, ident, pout_next, out_final, b2r, wregs,
                interleave=None):
    """One layer's edge phase: 49 dst tiles, per-segment pipelined."""
    ROW = ROW1 if lay < 2 else ROW2
    H = H12 if lay < 2 else H2
    C = C12 if lay < 2 else C2
    HC = H * C
    J1s, J2s, off16, offdl, offdlr, J1M, J2M = meta
    for t in range(NTILE):
        nreal = min(128, NS - t * 128)
        J1, J2 = J1s[t], J2s[t]
        NSUB = J1 + J2
        o16, odl, odlr = off16[t], offdl[t], offdlr[t]

        adt = sb.tile([128, 8], BF16, tag="eadt")
        nc.sync.dma_start(out=adt[:, :H], in_=adsl[t * 128:(t + 1) * 128, :H])

        # ---- self-loop column: direct DMA from the local table slice ----
        Gs = sb.tile([128, ROW], BF16, tag="eGs")
        if t * 128 + 128 <= SPL:
            nc.sync.dma_start(out=Gs[:], in_=tblA[t * 128:t * 128 + 128, :])
        else:
            r0 = t * 128 - SPL
            k = min(128, NSB - r0)
            nc.sync.dma_start(out=Gs[:k, :], in_=tblB[r0:r0 + k, :])
        sEs = sb.tile([128, 8], F32, tag="esEs")
        nc.vector.tensor_add(out=sEs[:, :H], in0=Gs[:, HC:HC + H], in1=adt[:, :H])
        wbs = _leaky_exp_w(nc, sb, H, 1, sEs[:, :H], "ws")
        Gws = sb.tile([128, HC], BF16, tag="eGws")
        nc.vector.tensor_tensor(
            out=Gws[:].rearrange("p (h c) -> p h c", h=H),
            in0=Gs[:, 0:HC].rearrange("p (h c) -> p h c", h=H),
            in1=wbs[:].rearrange("p (h o) -> p h o", o=1).to_broadcast([128, H, C]),
            op=ALU.mult)
        pnum = ps.tile([128, HC], F32, space="PSUM", tag="pnum")
        pden = ps.tile([128, H], F32, space="PSUM", tag="pden")
        nc.tensor.matmul(out=pnum[:], lhsT=ident[:], rhs=Gws[:],
                         start=True, stop=False)
        nc.tensor.matmul(out=pden[:], lhsT=ident[:], rhs=wbs[:],
                         start=True, stop=False)

        # ---- gathered segments A (tabA) and B (tabB) ----
        dlr = sb.tile([128, (J1M + J2M) * 128], BF16, tag="edlr")
        nc.sync.dma_start(
            out=dlr[:, :NSUB * 128],
            in_=dlr_flat[0:1, odlr:odlr + NSUB * 128].to_broadcast([128, NSUB * 128]))

        segs = []
        col = 0
        for si, (tab, jseg, JM) in enumerate(((tabA, J1, J1M), (tabB, J2, J2M))):
            G = sb.tile([128, JM * ROW], BF16, tag=f"eG{si}")
            c0 = col
            for w in _call_widths(jseg):
                nc.gpsimd.dma_gather(
                    G[:, (col - c0) * ROW:(col - c0 + w) * ROW]
                        .rearrange("p (j r) -> p j r", r=ROW),
                    tab[:], idx16_all[:, o16 + col * 8:o16 + (col + w) * 8],
                    w * 128, wregs[w], ROW)
                col += w
            segs.append((G, jseg, c0))

        for si, (G, jseg, c0) in enumerate(segs):
            if jseg == 0:
                continue
            JM = J1M if si == 0 else J2M
            S = sb.tile([128, JM * 128], BF16, tag=f"eS{si}")
            nc.vector.tensor_tensor(
                out=S[:, :jseg * 128].rearrange("p (n f) -> p n f", f=128),
                in0=dl_all[:, odl + c0:odl + c0 + jseg]
                    .rearrange("p (n o) -> p n o", o=1).to_broadcast([128, jseg, 128]),
                in1=io[:].rearrange("p (o f) -> p o f", o=1).to_broadcast([128, jseg, 128]),
                op=ALU.is_equal)
            ST = sb.tile([128, JM * 128], BF16, tag=f"eST{si}")
            nc.vector.tensor_tensor(
                out=ST[:, :jseg * 128], in0=dlr[:, c0 * 128:(c0 + jseg) * 128],
                in1=ioc[:].to_broadcast([128, jseg * 128]), op=ALU.is_equal)

            pad_ = ps.tile([128, JM * H], F32, space="PSUM", tag=f"pad{si}")
            for j in range(jseg):
                nc.tensor.matmul(out=pad_[:, H * j:H * (j + 1)],
                                 lhsT=ST[:, j * 128:(j + 1) * 128],
                                 rhs=adt[:, :H], start=True, stop=True)

            sE = sb.tile([128, JM * H], F32, tag=f"esE{si}")
            nc.vector.tensor_tensor(
                out=sE[:, :jseg * H].rearrange("p (n f) -> p n f", f=H),
                in0=G[:, :jseg * ROW].rearrange("p (n e) -> p n e", e=ROW)[:, :, HC:HC + H],
                in1=pad_[:, :jseg * H].rearrange("p (n f) -> p n f", f=H),
                op=ALU.add)
            wb = _leaky_exp_w(nc, sb, H, jseg, sE[:, :jseg * H], f"w{si}")

            Gw = sb.tile([128, JM * HC], BF16, tag=f"eGw{si}")
            nc.vector.tensor_tensor(
                out=Gw[:, :jseg * HC].rearrange("p (n h c) -> p n h c", h=H, c=C),
                in0=G[:, :jseg * ROW].rearrange("p (n e) -> p n e", e=ROW)[:, :, 0:HC]
                     .rearrange("p n (h c) -> p n h c", h=H),
                in1=wb[:, :jseg * H].rearrange("p (n h o) -> p n h o", h=H, o=1)
                     .to_broadcast([128, jseg, H, C]),
                op=ALU.mult)

            last = (si == (1 if J2 > 0 else 0))
            for j in range(jseg):
                nc.tensor.matmul(out=pnum[:], lhsT=S[:, j * 128:(j + 1) * 128],
                                 rhs=Gw[:, j * HC:(j + 1) * HC],
                                 start=False, stop=last and (j == jseg - 1))
                nc.tensor.matmul(out=pden[:], lhsT=S[:, j * 128:(j + 1) * 128],
                                 rhs=wb[:, j * H:(j + 1) * H],
                                 start=False, stop=last and (j == jseg - 1))

        rden = sb.tile([128, H], F32, tag="erden")
        nc.vector.reciprocal(out=rden[:], in_=pden[:])

        if lay < 2:
            ot = sb.tile([128, 512], BF16, tag="eot")
            for hh in range(H):
                nc.scalar.activation(out=ot[:, hh * C:(hh + 1) * C],
                                     in_=pnum[:, hh * C:(hh + 1) * C],
                                     func=AF.Identity, scale=rden[:, hh:hh + 1])
            ck, cr = (t * 128) // 512, (t * 128) % 512
            nc.sync.dma_start(out=pout_next[ck][cr:cr + nreal, :], in_=ot[:nreal, :])
        else:
            tmp = sb.tile([128, 240], F32, tag="etmp")
            for hh in range(H):
                nc.scalar.activation(out=tmp[:, hh * C:(hh + 1) * C],
                                     in_=pnum[:, hh * C:(hh + 1) * C],
                                     func=AF.Identity, scale=rden[:, hh:hh + 1])
            acc = sb.tile([128, 40], F32, tag="eacc")
            nc.vector.tensor_add(out=acc[:], in0=tmp[:, 0:40], in1=tmp[:, 40:80])
            for hh in range(2, 6):
                nc.vector.tensor_add(out=acc[:], in0=acc[:], in1=tmp[:, hh * 40:(hh + 1) * 40])
            z = sb.tile([128, 40], F32, tag="ez")
            nc.vector.tensor_scalar_mul(out=z[:], in0=acc[:], scalar1=1.0 / 6.0)
            nc.vector.tensor_add(out=z[:], in0=z[:], in1=b2r[:])
            mx = sb.tile([128, 1], F32, tag="emx")
            nc.vector.reduce_max(out=mx[:], in_=z[:], axis=mybir.AxisListType.X)
            zs = sb.tile([128, 40], F32, tag="ezs")
            nc.vector.tensor_scalar(out=zs[:], in0=z[:], scalar1=mx[:, 0:1], scalar2=None,
                                    op0=ALU.subtract)
            ex = sb.tile([128, 40], F32, tag="eex")
            nc.scalar.activation(out=ex[:], in_=zs[:], func=AF.Exp)
            sm = sb.tile([128, 1], F32, tag="esm")
            nc.vector.reduce_sum(out=sm[:], in_=ex[:], axis=mybir.AxisListType.X)
            ln = sb.tile([128, 1], F32, tag="eln")
            nc.scalar.activation(out=ln[:], in_=sm[:], func=AF.Ln)
            lsm = sb.tile([128, 40], F32, tag="elsm")
            nc.vector.tensor_scalar(out=lsm[:], in0=zs[:], scalar1=ln[:, 0:1], scalar2=None,
                                    op0=ALU.subtract)
            nc.sync.dma_start(out=out_final[t * 128:t * 128 + nreal, :], in_=lsm[:nreal, :])

        if interleave is not None:
            for cb in interleave.get(t, ()):
                cb()


def _node_chunk(nc, sb, ps, lay, it, xin, pout, wt, blk, bn_sc, bn_sh,
                tblA, tblB, adsl):
    """Emit one node-phase chunk (512 rows) of layer `lay`."""
    if lay == 0:
        d_in, d_out, H, HC = 128, 512, 4, 512
    elif lay == 1:
        d_in, d_out, H, HC = 512, 512, 4, 512
    else:
        d_in, d_out, H, HC = 512, 256, 6, 240
    NFB = d_in // 128
    n0 = it * 512
    aT = []
    for fb in range(NFB):
        if lay == 0:
            aT.append(xin[:, n0:n0 + 512])
            continue
        a = sb.tile([128, 512], BF16, tag=f"naT{fb}")
        zb = sb.tile([128, 512], BF16, tag="nzb")
        nc.sync.dma_start(out=zb[:], in_=pout[it][:, fb * 128:(fb + 1) * 128],
                          transpose=True)
        z = sb.tile([128, 512], F32, tag="nzf")
        nc.scalar.activation(out=z[:], in_=zb[:], func=AF.Identity,
                             bias=bn_sh[:, fb:fb + 1], scale=bn_sc[:, fb:fb + 1])
        mm = sb.tile([128, 512], F32, tag="nmm")
        nc.vector.tensor_scalar_min(out=mm[:], in0=z[:], scalar1=0.0)
        ee = sb.tile([128, 512], F32, tag="nee")
        nc.scalar.activation(out=ee[:], in_=mm[:], func=AF.Exp)
        rr = sb.tile([128, 512], F32, tag="nrr")
        nc.vector.scalar_tensor_tensor(out=rr[:], in0=z[:], scalar=0.0, in1=ee[:],
                                       op0=ALU.max, op1=ALU.add)
        nc.vector.tensor_scalar(out=a[:], in0=rr[:], scalar1=-1.0, scalar2=None,
                                op0=ALU.add)
        aT.append(a[:])
    for nb in range(4):
        ph = ps.tile([128, d_out], F32, space="PSUM", tag="nph")
        pa = ps.tile([128, 2 * H], F32, space="PSUM", tag="npa")
        for fb in range(NFB):
            lhsT = aT[fb][:, nb * 128:(nb + 1) * 128]
            nc.tensor.matmul(out=ph[:], lhsT=lhsT,
                             rhs=wt[:, fb * blk:fb * blk + d_out],
                             start=(fb == 0), stop=(fb == NFB - 1))
            nc.tensor.matmul(out=pa[:], lhsT=lhsT,
                             rhs=wt[:, fb * blk + d_out:(fb + 1) * blk],
                             start=(fb == 0), stop=(fb == NFB - 1))
        row0 = n0 + nb * 128
        nreal = min(128, max(0, NS - row0))
        if nreal == 0:
            continue
        used = HC + H if lay < 2 else ROW2
        tb = sb.tile([128, used], BF16, tag="ntb")
        if lay < 2:
            nc.vector.tensor_copy(out=tb[:, 0:512], in_=ph[:])
        else:
            nc.vector.tensor_copy(out=tb[:, 0:120], in_=ph[:, 0:120])
            nc.vector.tensor_copy(out=tb[:, 120:240], in_=ph[:, 128:248])
        nc.vector.tensor_copy(out=tb[:, HC:HC + H].rearrange("p (n o) -> p n o", o=1),
                              in_=pa[:].rearrange("p (n s) -> p n s", s=2)[:, :, 0:1])
        if lay == 2:
            nc.vector.tensor_copy(
                out=tb[:, HC + H:HC + 2 * H].rearrange("p (n o) -> p n o", o=1),
                in_=pa[:].rearrange("p (n s) -> p n s", s=2)[:, :, 1:2])
        if row0 + nreal <= SPL:
            nc.sync.dma_start(out=tblA[row0:row0 + nreal, 0:used], in_=tb[:nreal, :])
        elif row0 >= SPL:
            nc.sync.dma_start(out=tblB[row0 - SPL:row0 - SPL + nreal, 0:used],
                              in_=tb[:nreal, :])
        else:
            k = SPL - row0
            nc.sync.dma_start(out=tblA[row0:SPL, 0:used], in_=tb[:k, :])
            nc.sync.dma_start(out=tblB[0:nreal - k, 0:used], in_=tb[k:nreal, :])
        ab = sb.tile([128, 8], BF16, tag="nab")
        nc.vector.tensor_copy(out=ab[:, :H].rearrange("p (n o) -> p n o", o=1),
                              in_=pa[:].rearrange("p (n s) -> p n s", s=2)[:, :, 1:2])
        nc.sync.dma_start(out=adsl[row0:row0 + nreal, :H], in_=ab[:nreal, :H])


def _build(meta):
    J1s, J2s, off16, offdl, offdlr, J1M, J2M = meta
    TOT16 = off16[-1]
    TOTDL = offdl[-1]
    TOTDLR = offdlr[-1]
    nc = bass.Bass()
    # ---- inputs (per-core) ----
    xT = nc.declare_dram_parameter("xT", [128, PADN], BF16, isOutput=False)
    W0 = nc.declare_dram_parameter("W0p", [128, 512 + 8], BF16, isOutput=False)
    W1 = nc.declare_dram_parameter("W1p", [128, 4 * (512 + 8)], BF16, isOutput=False)
    W2 = nc.declare_dram_parameter("W2p", [128, 4 * (256 + 12)], BF16, isOutput=False)
    bnsc0 = nc.declare_dram_parameter("bnsc0", [128, 4], F32, isOutput=False)
    bnsh0 = nc.declare_dram_parameter("bnsh0", [128, 4], F32, isOutput=False)
    bnsc1 = nc.declare_dram_parameter("bnsc1", [128, 4], F32, isOutput=False)
    bnsh1 = nc.declare_dram_parameter("bnsh1", [128, 4], F32, isOutput=False)
    b2r = nc.declare_dram_parameter("b2r", [128, 40], F32, isOutput=False)
    iota = nc.declare_dram_parameter("iota", [128, 128], BF16, isOutput=False)
    iotac = nc.declare_dram_parameter("iotac", [128, 1], BF16, isOutput=False)
    eidx = nc.declare_dram_parameter("eidx", [128, TOT16], I16, isOutput=False)
    edl = nc.declare_dram_parameter("edl", [128, TOTDL], BF16, isOutput=False)
    edlr = nc.declare_dram_parameter("edlr", [1, TOTDLR], BF16, isOutput=False)
    out = nc.declare_dram_parameter("out", [NS, 40], F32, isOutput=True)
    # ---- internal ----
    tblA0 = nc.dram_tensor("tblA0", [SPL, ROW1], BF16)
    tblB0 = nc.dram_tensor("tblB0", [NSB, ROW1], BF16)
    tblA1 = nc.dram_tensor("tblA1", [SPL, ROW1], BF16)
    tblB1 = nc.dram_tensor("tblB1", [NSB, ROW1], BF16)
    tblA2 = nc.dram_tensor("tblA2", [SPL, ROW2], BF16)
    tblB2 = nc.dram_tensor("tblB2", [NSB, ROW2], BF16)
    tabA0 = nc.dram_tensor("tabA0", [NC * SPL, ROW1], BF16, addr_space="Shared")
    tabB0 = nc.dram_tensor("tabB0", [NC * NSB, ROW1], BF16, addr_space="Shared")
    tabA1 = nc.dram_tensor("tabA1", [NC * SPL, ROW1], BF16, addr_space="Shared")
    tabB1 = nc.dram_tensor("tabB1", [NC * NSB, ROW1], BF16, addr_space="Shared")
    tabA2 = nc.dram_tensor("tabA2", [NC * SPL, ROW2], BF16, addr_space="Shared")
    tabB2 = nc.dram_tensor("tabB2", [NC * NSB, ROW2], BF16, addr_space="Shared")
    pout0 = [nc.dram_tensor(f"pout0_{k}", [512, 512], BF16) for k in range(NCHUNK)]
    pout1 = [nc.dram_tensor(f"pout1_{k}", [512, 512], BF16) for k in range(NCHUNK)]
    adsl0 = nc.dram_tensor("adsl0", [NTILE * 128, 8], BF16)
    adsl1 = nc.dram_tensor("adsl1", [NTILE * 128, 8], BF16)
    adsl2 = nc.dram_tensor("adsl2", [NTILE * 128, 8], BF16)

    rg = [list(range(NC))]
    with tile.TileContext(nc) as tc:
        with tc.tile_pool(name="cst", bufs=1) as cst, \
             tc.tile_pool(name="sb", bufs=2) as sb, \
             tc.tile_pool(name="ps", bufs=1, space="PSUM") as ps:
            io = cst.tile([128, 128], BF16)
            nc.sync.dma_start(out=io[:], in_=iota[:])
            ioc = cst.tile([128, 1], BF16)
            nc.sync.dma_start(out=ioc[:], in_=iotac[:])
            ident = cst.tile([128, 128], BF16)
            nc.vector.tensor_tensor(out=ident[:], in0=ioc[:].to_broadcast([128, 128]),
                                    in1=io[:], op=ALU.is_equal)
            b2 = cst.tile([128, 40], F32)
            nc.sync.dma_start(out=b2[:], in_=b2r[:])
            xt = cst.tile([128, PADN], BF16)
            nc.sync.dma_start(out=xt[:], in_=xT[:])
            w0 = cst.tile([128, 520], BF16)
            nc.sync.dma_start(out=w0[:], in_=W0[:])
            w1 = cst.tile([128, 4 * 520], BF16)
            nc.sync.dma_start(out=w1[:], in_=W1[:])
            w2 = cst.tile([128, 4 * 268], BF16)
            nc.sync.dma_start(out=w2[:], in_=W2[:])
            sc0 = cst.tile([128, 4], F32); nc.sync.dma_start(out=sc0[:], in_=bnsc0[:])
            sh0 = cst.tile([128, 4], F32); nc.sync.dma_start(out=sh0[:], in_=bnsh0[:])
            sc1 = cst.tile([128, 4], F32); nc.sync.dma_start(out=sc1[:], in_=bnsc1[:])
            sh1 = cst.tile([128, 4], F32); nc.sync.dma_start(out=sh1[:], in_=bnsh1[:])
            idx16_all = cst.tile([128, TOT16], I16)
            nc.sync.dma_start(out=idx16_all[:], in_=eidx[:])
            dl_all = cst.tile([128, TOTDL], BF16)
            nc.sync.dma_start(out=dl_all[:], in_=edl[:])
            zz = cst.tile([128, 512], BF16)
            nc.gpsimd.memset(zz[:], 0.0)
            wregs = {w: nc.gpsimd.to_reg(w * 128)
                     for w in set(sum((_call_widths(j) for j in J1s + J2s), []))}
            # zero pad tail of last pout chunk (rows 6250.. of global layout)
            for pout in (pout0, pout1):
                r = NS - 12 * 512
                while r < 512:
                    k = min(128, 512 - r)
                    nc.sync.dma_start(out=pout[NCHUNK - 1][r:r + k, :], in_=zz[:k, :])
                    r += k
            for adsl in (adsl0, adsl1, adsl2):
                nc.sync.dma_start(out=adsl[NS:NTILE * 128, :], in_=zz[:NTILE * 128 - NS, :8])

            def ag(tbl, tab):
                return lambda: nc.gpsimd.collective_compute(
                    "AllGather", ALU.bypass, replica_groups=rg,
                    ins=[tbl[:]], outs=[tab[:]])

            def nodes(lay, pout, wt, blk, sc, sh, tblA, tblB, adsl):
                def mk(k):
                    return lambda: _node_chunk(nc, sb, ps, lay, k, None, pout,
                                               wt, blk, sc, sh, tblA, tblB, adsl)
                return {4 * k + 3: [mk(k)] for k in range(12)} | {48: [mk(12)]}

            # Layer 0 node phase (inputs are resident; AG-A once rows<SPL done)
            for k in range(13):
                _node_chunk(nc, sb, ps, 0, k, xt[:], None, w0[:], 520, None, None,
                            tblA0, tblB0, adsl0)
                if k == 5:
                    ag(tblA0, tabA0)()
            ag(tblB0, tabB0)()

            il1 = nodes(1, pout0, w1[:], 520, sc0[:], sh0[:], tblA1, tblB1, adsl1)
            il1[23] = il1.get(23, []) + [ag(tblA1, tabA1)]
            il1[48] = il1.get(48, []) + [ag(tblB1, tabB1)]
            _edge_phase(nc, sb, ps, 0, meta, tabA0, tabB0, tblA0, tblB0, idx16_all,
                        dl_all, edlr, adsl0, io, ioc, ident, pout0, None, None,
                        wregs, il1)

            il2 = nodes(2, pout1, w2[:], 268, sc1[:], sh1[:], tblA2, tblB2, adsl2)
            il2[23] = il2.get(23, []) + [ag(tblA2, tabA2)]
            il2[48] = il2.get(48, []) + [ag(tblB2, tabB2)]
            _edge_phase(nc, sb, ps, 1, meta, tabA1, tabB1, tblA1, tblB1, idx16_all,
                        dl_all, edlr, adsl1, io, ioc, ident, pout1, None, None,
                        wregs, il2)

            _edge_phase(nc, sb, ps, 2, meta, tabA2, tabB2, tblA2, tblB2, idx16_all,
                        dl_all, edlr, adsl2, io, ioc, ident, None, out, b2[:], wregs)
    _hoist_waits(nc)
    mask = {}
    for lib in all_libraries:
        for ty in lib.instructions:
            mask[ty] = mask.get(ty, 0) | (1 << lib.index)
    bass_rust.insert_library_loads(nc, mask, len(all_libraries), standard.index)
    mybir.codegen_inst_isa_subclasses(nc)
    return nc


def _prep_edges(edge_index):
    """Per-core edge arrays for dma_gather (self-loops excluded).

    Returns (meta, per-core (idx16, dl, dlr) arrays) where meta =
    (J1s, J2s, off16, offdl, offdlr, J1M, J2M); offsets have NTILE+1
    entries (element counts into the flat arrays).
    """
    src = edge_index[0].astype(np.int64)
    dst = edge_index[1].astype(np.int64)
    core = dst // NS
    tloc = (dst % NS) // 128
    s_loc = src % NS
    s_core = src // NS
    inA = s_loc < SPL
    rowA = s_core * SPL + s_loc
    rowB = s_core * NSB + (s_loc - SPL)
    percore = []
    for c in range(NC):
        m = core == c
        d_c, t_c = dst[m], tloc[m]
        a_c, ra_c, rb_c = inA[m], rowA[m], rowB[m]
        dl_c = (d_c % NS) - t_c * 128
        tiles = []
        for t in range(NTILE):
            mt = t_c == t
            aa = a_c[mt]
            tiles.append((ra_c[mt][aa], dl_c[mt][aa],
                          rb_c[mt][~aa], dl_c[mt][~aa]))
        percore.append(tiles)
    J1s, J2s = [], []
    for t in range(NTILE):
        J1s.append(max(-(-len(percore[c][t][0]) // 128) for c in range(NC)))
        J2s.append(max(-(-len(percore[c][t][2]) // 128) for c in range(NC)))
    J1M, J2M = max(J1s), max(J2s)
    off16 = [0]; offdl = [0]; offdlr = [0]
    for t in range(NTILE):
        ns = J1s[t] + J2s[t]
        off16.append(off16[-1] + ns * 8)
        offdl.append(offdl[-1] + ns)
        offdlr.append(offdlr[-1] + ns * 128)
    meta = (J1s, J2s, off16, offdl, offdlr, J1M, J2M)
    arrs = []
    for c in range(NC):
        idx16 = np.zeros((16, off16[-1]), np.int16)
        dlf = np.full((128, offdl[-1]), 999.0, np.float32)
        dlrf = np.full((1, offdlr[-1]), 999.0, np.float32)
        for t in range(NTILE):
            rA, dA, rB, dB = percore[c][t]
            for (rr, dd, joff) in ((rA, dA, 0), (rB, dB, J1s[t])):
                e = np.arange(len(rr))
                idx16[e % 16, off16[t] + joff * 8 + e // 16] = rr.astype(np.int16)
                dlf[e % 128, offdl[t] + joff + e // 128] = dd
                dlrf[0, offdlr[t] + joff * 128 + (e // 128) * 128 + e % 128] = dd
        arrs.append((np.tile(idx16, (8, 1)), dlf.astype(bf), dlrf.astype(bf)))
    return meta, arrs


def kernel(x, edge_index, W0, as0, ad0, b0, g0, bt0, m0, v0,
           W1, as1, ad1, b1, g1, bt1, m1, v1,
           W2, as2, ad2, b2):
    x = np.asarray(x, np.float32)
    meta, earrs = _prep_edges(np.asarray(edge_index))

    # host-side packing
    def packW(W, a_s, a_d, d_out_pad, Hh, Cc, headpad):
        # W: [d_in, d_out]; returns [128, NFB*(d_out_pad + 2H)] bf16
        d_in = W.shape[0]
        NFB = d_in // 128
        if headpad:  # L2: pad head layout (3 heads of 40 per 128-block)
            Wp = np.zeros((d_in, 256), np.float32)
            Wp[:, 0:120] = W[:, 0:120]
            Wp[:, 128:248] = W[:, 120:240]
        else:
            Wp = W.astype(np.float32)
        Wa = np.zeros((d_in, 2 * Hh), np.float32)
        for h in range(Hh):
            Wa[:, 2 * h] = W[:, h * Cc:(h + 1) * Cc] @ a_s[h]
            Wa[:, 2 * h + 1] = W[:, h * Cc:(h + 1) * Cc] @ a_d[h]
        blk = np.concatenate([Wp, Wa], axis=1)          # [d_in, d_out_pad+2H]
        blk = blk.reshape(NFB, 128, -1)
        return np.concatenate([blk[i] for i in range(NFB)], axis=1).astype(bf)

    W0 = np.asarray(W0, np.float32); W1 = np.asarray(W1, np.float32); W2 = np.asarray(W2, np.float32)
    as0 = np.asarray(as0, np.float32); ad0 = np.asarray(ad0, np.float32)
    as1 = np.asarray(as1, np.float32); ad1 = np.asarray(ad1, np.float32)
    as2 = np.asarray(as2, np.float32); ad2 = np.asarray(ad2, np.float32)
    W0p = packW(W0, as0, ad0, 512, 4, 128, False)
    W1p = packW(W1, as1, ad1, 512, 4, 128, False)
    W2p = packW(W2, as2, ad2, 256, 6, 40, True)

    def bnfold(g, bt, m, v, b):
        sc = np.asarray(g, np.float32) / np.sqrt(np.asarray(v, np.float32) + BN_EPS)
        sh = (np.asarray(b, np.float32) - np.asarray(m, np.float32)) * sc + np.asarray(bt, np.float32)
        return sc.reshape(4, 128).T.copy(), sh.reshape(4, 128).T.copy()
    sc0, sh0 = bnfold(g0, bt0, m0, v0, b0)
    sc1, sh1 = bnfold(g1, bt1, m1, v1, b1)
    b2rep = np.broadcast_to(np.asarray(b2, np.float32), (128, 40)).copy()

    iota = np.broadcast_to(np.arange(128, dtype=np.float32), (128, 128)).astype(bf)
    iotac = np.arange(128, dtype=np.float32)[:, None].astype(bf)

    xT = x.T.astype(bf)   # [128, N]
    in_maps = []
    for c in range(NC):
        xts = np.zeros((128, PADN), bf)
        xts[:, :NS] = xT[:, c * NS:(c + 1) * NS]
        idx16, dlf, dlrf = earrs[c]
        in_maps.append({
            "xT": xts, "W0p": W0p, "W1p": W1p, "W2p": W2p,
            "bnsc0": sc0, "bnsh0": sh0, "bnsc1": sc1, "bnsh1": sh1,
            "b2r": b2rep, "iota": iota, "iotac": iotac,
            "eidx": idx16, "edl": dlf, "edlr": dlrf,
        })

    global _last_in_maps, _last_meta
    _last_in_maps = in_maps
    _last_meta = meta
    nc = _build(meta)
    res = run_bass_kernel_spmd(nc, in_maps, core_ids=list(range(NC)))
    out = np.concatenate([res.results[c]["out"] for c in range(NC)], axis=0)
    return out.astype(np.float32)


# revision 25
# speedup vs baseline: 1.6600x; 1.0721x over previous
"""3-layer GAT (arXiv-style) on 8 Trainium2 NeuronCores via Bass.

Sharding: dst-node sharding (6250 nodes/core). Node phase computes each
core's h-table slice [h | a_src-dot] rows, split at local row 2688 into two
tables; two AllGathers replicate them (each <32768 rows so dma_gather's
int16 indices reach every row). Edge phase gathers per-edge source rows
with gpsimd dma_gather (<=1024 indices/call, per-tile exact widths),
handles self-loops via a direct DMA + identity matmul, builds 0/1
selection matrices from dst-locals vs iota, and does segment-softmax +
feature aggregation as PSUM-accumulated matmuls, pipelined per segment.
The next layer's node chunks and the first AllGather are interleaved into
the edge tile loop so collectives overlap gathers. Output: log_softmax.
"""
import numpy as np
import ml_dtypes

import concourse.bass as bass
import concourse.mybir as mybir
import concourse.tile as tile
from concourse.bass_utils import run_bass_kernel_spmd
from concourse.library_config import all_libraries, standard
import bass_rust

# ---- problem constants (hardcoded per harness contract) ----
N = 50000
E = 800000
F_IN = 128
NEG = 0.2
BN_EPS = 1e-5
NC = 8
NS = N // NC            # 6250 nodes per core
NTILE = (NS + 127) // 128   # 49 dst tiles per core
PADN = 13 * 512         # node-phase padded slice rows (6656)
NCHUNK = 13             # pout row chunks (512 rows each) for overlap
SPL = 2688              # local-row split: tblA rows [0,2688), tblB [2688,6250)
NSB = NS - SPL          # 3562 rows per core in tblB
ROW1 = 640              # layer0/1 table row: h(512)+as(4)+pad (1280B, %256)
ROW2 = 256              # layer2 table row: h(240)+as(6)+ad(6)+pad (512B, %256)
H12, C12 = 4, 128
H2, C2 = 6, 40
MAXW = 8                # dma_gather call width cap (1024-descriptor ring)
AF = mybir.ActivationFunctionType
ALU = mybir.AluOpType
dt = mybir.dt
F32, BF16, I16 = dt.float32, dt.bfloat16, dt.int16
bf = ml_dtypes.bfloat16


def _hoist_waits(nc, max_keep=1):
    n = 0
    for f in nc.m.functions:
        for bb in f.blocks:
            out, changed = [], False
            for ins in bb.instructions:
                si = getattr(ins, "sync_info", None)
                if si is not None and si.on_wait:
                    keep = 0 if (isinstance(ins, mybir.InstDMAGatherAnt)
                                 or (isinstance(ins, mybir.InstDMACopy)
                                     and getattr(ins, "queue", None) == "qPoolDynamic")) else max_keep
                    waits = list(si.on_wait)
                    if len(waits) > keep:
                        cut = len(waits) - keep
                        for w in waits[:cut]:
                            out.append(mybir.InstEventSemaphore(
                                name=f"I-hw-{n}", engine=ins.engine, ins=[], outs=[],
                                sync_info=mybir.SyncInfo(on_wait=[w], on_update=[])))
                            n += 1
                        si.on_wait = waits[cut:]
                        changed = True
                out.append(ins)
            if changed:
                bb.instructions = out
    return n


def _call_widths(jseg):
    """Split a segment of jseg 128-idx columns into balanced calls of <=MAXW."""
    if jseg == 0:
        return []
    ncalls = -(-jseg // MAXW)
    base = jseg // ncalls
    rem = jseg - base * ncalls
    return [base + (1 if i < rem else 0) for i in range(ncalls)]


def _leaky_exp_w(nc, sb, H, nsub, sE, tag):
    """wb = exp(leaky_relu(sE)) in bf16; sE is a f32 AP [128, nsub*H]."""
    m2 = sb.tile([128, nsub * H], F32, tag=tag + "m")
    nc.vector.tensor_scalar(out=m2[:], in0=sE, scalar1=0.0, scalar2=NEG,
                            op0=ALU.min, op1=ALU.mult)
    lr = sb.tile([128, nsub * H], F32, tag=tag + "l")
    nc.vector.scalar_tensor_tensor(out=lr[:], in0=sE, scalar=0.0, in1=m2[:],
                                   op0=ALU.max, op1=ALU.add)
    wb = sb.tile([128, nsub * H], BF16, tag=tag + "b")
    nc.scalar.activation(out=wb[:], in_=lr[:], func=AF.Exp)
    return wb


def _edge_phase(nc, sb, ps, lay, meta, tabA, tabB, tblA, tblB, idx16_all, dl_all,
                dlr_flat, adsl, io, ioc, ident, pout_next, out_final, b2r, wregs,
                interleave=None):
    """One layer's edge phase: 49 dst tiles, per-segment pipelined."""
    ROW = ROW1 if lay < 2 else ROW2
    H = H12 if lay < 2 else H2
    C = C12 if lay < 2 else C2
    HC = H * C
    J1s, J2s, off16, offdl, offdlr, J1M, J2M = meta
    for t in range(NTILE):
        nreal = min(128, NS - t * 128)
        J1, J2 = J1s[t], J2s[t]
        NSUB = J1 + J2
        o16, odl, odlr = off16[t], offdl[t], offdlr[t]

        adt = sb.tile([128, 8], BF16, tag="eadt")
        nc.sync.dma_start(out=adt[:, :H], in_=adsl[t * 128:(t + 1) * 128, :H])

        # ---- self-loop column: direct DMA from the local table slice ----
        Gs = sb.tile([128, ROW], BF16, tag="eGs")
        if t * 128 + 128 <= SPL:
            nc.sync.dma_start(out=Gs[:], in_=tblA[t * 128:t * 128 + 128, :])
        else:
            r0 = t * 128 - SPL
            k = min(128, NSB - r0)
            nc.sync.dma_start(out=Gs[:k, :], in_=tblB[r0:r0 + k, :])

        # ---- gathered segments A (tabA) and B (tabB) ----
        dlr = sb.tile([128, (J1M + J2M) * 128], BF16, tag="edlr")
        nc.sync.dma_start(
            out=dlr[:, :NSUB * 128],
            in_=dlr_flat[0:1, odlr:odlr + NSUB * 128].to_broadcast([128, NSUB * 128]))

        segs = []
        col = 0
        for si, (tab, jseg, JM) in enumerate(((tabA, J1, J1M), (tabB, J2, J2M))):
            G = sb.tile([128, JM * ROW], BF16, tag=f"eG{si}", bufs=3)
            c0 = col
            for w in _call_widths(jseg):
                nc.gpsimd.dma_gather(
                    G[:, (col - c0) * ROW:(col - c0 + w) * ROW]
                        .rearrange("p (j r) -> p j r", r=ROW),
                    tab[:], idx16_all[:, o16 + col * 8:o16 + (col + w) * 8],
                    w * 128, wregs[w], ROW)
                col += w
            segs.append((G, jseg, c0))

        pnum = ps.tile([128, HC], F32, space="PSUM", tag="pnum")
        pden = ps.tile([128, H], F32, space="PSUM", tag="pden")
        for si, (G, jseg, c0) in enumerate(segs):
            if jseg == 0:
                continue
            JM = J1M if si == 0 else J2M
            S = sb.tile([128, JM * 128], BF16, tag=f"eS{si}")
            nc.vector.tensor_tensor(
                out=S[:, :jseg * 128].rearrange("p (n f) -> p n f", f=128),
                in0=dl_all[:, odl + c0:odl + c0 + jseg]
                    .rearrange("p (n o) -> p n o", o=1).to_broadcast([128, jseg, 128]),
                in1=io[:].rearrange("p (o f) -> p o f", o=1).to_broadcast([128, jseg, 128]),
                op=ALU.is_equal)
            ST = sb.tile([128, JM * 128], BF16, tag=f"eST{si}")
            nc.vector.tensor_tensor(
                out=ST[:, :jseg * 128], in0=dlr[:, c0 * 128:(c0 + jseg) * 128],
                in1=ioc[:].to_broadcast([128, jseg * 128]), op=ALU.is_equal)

            pad_ = ps.tile([128, JM * H], F32, space="PSUM", tag=f"pad{si}")
            for j in range(jseg):
                nc.tensor.matmul(out=pad_[:, H * j:H * (j + 1)],
                                 lhsT=ST[:, j * 128:(j + 1) * 128],
                                 rhs=adt[:, :H], start=True, stop=True)

            # segment A also carries the self-loop column at the tail
            EX = 1 if si == 0 else 0
            sE = sb.tile([128, (JM + 1) * H], F32, tag=f"esE{si}")
            nc.vector.tensor_tensor(
                out=sE[:, :jseg * H].rearrange("p (n f) -> p n f", f=H),
                in0=G[:, :jseg * ROW].rearrange("p (n e) -> p n e", e=ROW)[:, :, HC:HC + H],
                in1=pad_[:, :jseg * H].rearrange("p (n f) -> p n f", f=H),
                op=ALU.add)
            if EX:
                nc.vector.tensor_add(out=sE[:, jseg * H:(jseg + 1) * H],
                                     in0=Gs[:, HC:HC + H], in1=adt[:, :H])
            wb = _leaky_exp_w(nc, sb, H, jseg + EX, sE[:, :(jseg + EX) * H], f"w{si}")

            Gw = sb.tile([128, JM * HC], BF16, tag=f"eGw{si}")
            nc.vector.tensor_tensor(
                out=Gw[:, :jseg * HC].rearrange("p (n h c) -> p n h c", h=H, c=C),
                in0=G[:, :jseg * ROW].rearrange("p (n e) -> p n e", e=ROW)[:, :, 0:HC]
                     .rearrange("p n (h c) -> p n h c", h=H),
                in1=wb[:, :jseg * H].rearrange("p (n h o) -> p n h o", h=H, o=1)
                     .to_broadcast([128, jseg, H, C]),
                op=ALU.mult)
            if EX:
                Gws = sb.tile([128, HC], BF16, tag="eGws")
                nc.vector.tensor_tensor(
                    out=Gws[:].rearrange("p (h c) -> p h c", h=H),
                    in0=Gs[:, 0:HC].rearrange("p (h c) -> p h c", h=H),
                    in1=wb[:, jseg * H:(jseg + 1) * H]
                        .rearrange("p (h o) -> p h o", o=1).to_broadcast([128, H, C]),
                    op=ALU.mult)
                nc.tensor.matmul(out=pnum[:], lhsT=ident[:], rhs=Gws[:],
                                 start=True, stop=False)
                nc.tensor.matmul(out=pden[:], lhsT=ident[:],
                                 rhs=wb[:, jseg * H:(jseg + 1) * H],
                                 start=True, stop=False)

            last = (si == (1 if J2 > 0 else 0))
            for j in range(jseg):
                nc.tensor.matmul(out=pnum[:], lhsT=S[:, j * 128:(j + 1) * 128],
                                 rhs=Gw[:, j * HC:(j + 1) * HC],
                                 start=False, stop=last and (j == jseg - 1))
                nc.tensor.matmul(out=pden[:], lhsT=S[:, j * 128:(j + 1) * 128],
                                 rhs=wb[:, j * H:(j + 1) * H],
                                 start=False, stop=last and (j == jseg - 1))

        rden = sb.tile([128, H], F32, tag="erden")
        nc.vector.reciprocal(out=rden[:], in_=pden[:])

        if lay < 2:
            ot = sb.tile([128, 512], BF16, tag="eot")
            for hh in range(H):
                nc.scalar.activation(out=ot[:, hh * C:(hh + 1) * C],
                                     in_=pnum[:, hh * C:(hh + 1) * C],
                                     func=AF.Identity, scale=rden[:, hh:hh + 1])
            ck, cr = (t * 128) // 512, (t * 128) % 512
            nc.sync.dma_start(out=pout_next[ck][cr:cr + nreal, :], in_=ot[:nreal, :])
        else:
            tmp = sb.tile([128, 240], F32, tag="etmp")
            for hh in range(H):
                nc.scalar.activation(out=tmp[:, hh * C:(hh + 1) * C],
                                     in_=pnum[:, hh * C:(hh + 1) * C],
                                     func=AF.Identity, scale=rden[:, hh:hh + 1])
            acc = sb.tile([128, 40], F32, tag="eacc")
            nc.vector.tensor_add(out=acc[:], in0=tmp[:, 0:40], in1=tmp[:, 40:80])
            for hh in range(2, 6):
                nc.vector.tensor_add(out=acc[:], in0=acc[:], in1=tmp[:, hh * 40:(hh + 1) * 40])
            z = sb.tile([128, 40], F32, tag="ez")
            nc.vector.tensor_scalar_mul(out=z[:], in0=acc[:], scalar1=1.0 / 6.0)
            nc.vector.tensor_add(out=z[:], in0=z[:], in1=b2r[:])
            mx = sb.tile([128, 1], F32, tag="emx")
            nc.vector.reduce_max(out=mx[:], in_=z[:], axis=mybir.AxisListType.X)
            zs = sb.tile([128, 40], F32, tag="ezs")
            nc.vector.tensor_scalar(out=zs[:], in0=z[:], scalar1=mx[:, 0:1], scalar2=None,
                                    op0=ALU.subtract)
            ex = sb.tile([128, 40], F32, tag="eex")
            nc.scalar.activation(out=ex[:], in_=zs[:], func=AF.Exp)
            sm = sb.tile([128, 1], F32, tag="esm")
            nc.vector.reduce_sum(out=sm[:], in_=ex[:], axis=mybir.AxisListType.X)
            ln = sb.tile([128, 1], F32, tag="eln")
            nc.scalar.activation(out=ln[:], in_=sm[:], func=AF.Ln)
            lsm = sb.tile([128, 40], F32, tag="elsm")
            nc.vector.tensor_scalar(out=lsm[:], in0=zs[:], scalar1=ln[:, 0:1], scalar2=None,
                                    op0=ALU.subtract)
            nc.sync.dma_start(out=out_final[t * 128:t * 128 + nreal, :], in_=lsm[:nreal, :])

        if interleave is not None:
            for cb in interleave.get(t, ()):
                cb()


def _node_chunk(nc, sb, ps, lay, it, xin, pout, wt, blk, bn_sc, bn_sh,
                tblA, tblB, adsl):
    """Emit one node-phase chunk (512 rows) of layer `lay`."""
    if lay == 0:
        d_in, d_out, H, HC = 128, 512, 4, 512
    elif lay == 1:
        d_in, d_out, H, HC = 512, 512, 4, 512
    else:
        d_in, d_out, H, HC = 512, 256, 6, 240
    NFB = d_in // 128
    n0 = it * 512
    aT = []
    for fb in range(NFB):
        if lay == 0:
            xa = sb.tile([128, 512], BF16, tag="nxa")
            nc.sync.dma_start(out=xa[:], in_=xin[:, n0:n0 + 512])
            aT.append(xa[:])
            continue
        a = sb.tile([128, 512], BF16, tag=f"naT{fb}")
        zb = sb.tile([128, 512], BF16, tag="nzb")
        nc.sync.dma_start(out=zb[:], in_=pout[it][:, fb * 128:(fb + 1) * 128],
                          transpose=True)
        z = sb.tile([128, 512], F32, tag="nzf")
        nc.scalar.activation(out=z[:], in_=zb[:], func=AF.Identity,
                             bias=bn_sh[:, fb:fb + 1], scale=bn_sc[:, fb:fb + 1])
        mm = sb.tile([128, 512], F32, tag="nmm")
        nc.scalar.activation(out=mm[:], in_=z[:], func=AF.Relu, scale=-1.0)
        ee = sb.tile([128, 512], F32, tag="nee")
        nc.scalar.activation(out=ee[:], in_=mm[:], func=AF.Exp, scale=-1.0)
        rr = sb.tile([128, 512], F32, tag="nrr")
        nc.vector.scalar_tensor_tensor(out=rr[:], in0=z[:], scalar=0.0, in1=ee[:],
                                       op0=ALU.max, op1=ALU.add)
        nc.vector.tensor_scalar(out=a[:], in0=rr[:], scalar1=-1.0, scalar2=None,
                                op0=ALU.add)
        aT.append(a[:])
    for nb in range(4):
        ph = ps.tile([128, d_out], F32, space="PSUM", tag="nph")
        pa = ps.tile([128, 2 * H], F32, space="PSUM", tag="npa")
        for fb in range(NFB):
            lhsT = aT[fb][:, nb * 128:(nb + 1) * 128]
            nc.tensor.matmul(out=ph[:], lhsT=lhsT,
                             rhs=wt[:, fb * blk:fb * blk + d_out],
                             start=(fb == 0), stop=(fb == NFB - 1))
            nc.tensor.matmul(out=pa[:], lhsT=lhsT,
                             rhs=wt[:, fb * blk + d_out:(fb + 1) * blk],
                             start=(fb == 0), stop=(fb == NFB - 1))
        row0 = n0 + nb * 128
        nreal = min(128, max(0, NS - row0))
        if nreal == 0:
            continue
        used = HC + H if lay < 2 else ROW2
        tb = sb.tile([128, used], BF16, tag="ntb")
        if lay < 2:
            nc.scalar.activation(out=tb[:, 0:512], in_=ph[:], func=AF.Identity)
        else:
            nc.scalar.activation(out=tb[:, 0:120], in_=ph[:, 0:120], func=AF.Identity)
            nc.scalar.activation(out=tb[:, 120:240], in_=ph[:, 128:248], func=AF.Identity)
        nc.vector.tensor_copy(out=tb[:, HC:HC + H].rearrange("p (n o) -> p n o", o=1),
                              in_=pa[:].rearrange("p (n s) -> p n s", s=2)[:, :, 0:1])
        if lay == 2:
            nc.vector.tensor_copy(
                out=tb[:, HC + H:HC + 2 * H].rearrange("p (n o) -> p n o", o=1),
                in_=pa[:].rearrange("p (n s) -> p n s", s=2)[:, :, 1:2])
        if row0 + nreal <= SPL:
            nc.sync.dma_start(out=tblA[row0:row0 + nreal, 0:used], in_=tb[:nreal, :])
        elif row0 >= SPL:
            nc.sync.dma_start(out=tblB[row0 - SPL:row0 - SPL + nreal, 0:used],
                              in_=tb[:nreal, :])
        else:
            k = SPL - row0
            nc.sync.dma_start(out=tblA[row0:SPL, 0:used], in_=tb[:k, :])
            nc.sync.dma_start(out=tblB[0:nreal - k, 0:used], in_=tb[k:nreal, :])
        ab = sb.tile([128, 8], BF16, tag="nab")
        nc.vector.tensor_copy(out=ab[:, :H].rearrange("p (n o) -> p n o", o=1),
                              in_=pa[:].rearrange("p (n s) -> p n s", s=2)[:, :, 1:2])
        nc.sync.dma_start(out=adsl[row0:row0 + nreal, :H], in_=ab[:nreal, :H])


def _build(meta):
    J1s, J2s, off16, offdl, offdlr, J1M, J2M = meta
    TOT16 = off16[-1]
    TOTDL = offdl[-1]
    TOTDLR = offdlr[-1]
    nc = bass.Bass()
    # ---- inputs (per-core) ----
    xT = nc.declare_dram_parameter("xT", [128, PADN], BF16, isOutput=False)
    W0 = nc.declare_dram_parameter("W0p", [128, 512 + 8], BF16, isOutput=False)
    W1 = nc.declare_dram_parameter("W1p", [128, 4 * (512 + 8)], BF16, isOutput=False)
    W2 = nc.declare_dram_parameter("W2p", [128, 4 * (256 + 12)], BF16, isOutput=False)
    bnsc0 = nc.declare_dram_parameter("bnsc0", [128, 4], F32, isOutput=False)
    bnsh0 = nc.declare_dram_parameter("bnsh0", [128, 4], F32, isOutput=False)
    bnsc1 = nc.declare_dram_parameter("bnsc1", [128, 4], F32, isOutput=False)
    bnsh1 = nc.declare_dram_parameter("bnsh1", [128, 4], F32, isOutput=False)
    b2r = nc.declare_dram_parameter("b2r", [128, 40], F32, isOutput=False)
    iota = nc.declare_dram_parameter("iota", [128, 128], BF16, isOutput=False)
    iotac = nc.declare_dram_parameter("iotac", [128, 1], BF16, isOutput=False)
    eidx = nc.declare_dram_parameter("eidx", [128, TOT16], I16, isOutput=False)
    edl = nc.declare_dram_parameter("edl", [128, TOTDL], BF16, isOutput=False)
    edlr = nc.declare_dram_parameter("edlr", [1, TOTDLR], BF16, isOutput=False)
    out = nc.declare_dram_parameter("out", [NS, 40], F32, isOutput=True)
    # ---- internal ----
    tblA0 = nc.dram_tensor("tblA0", [SPL, ROW1], BF16)
    tblB0 = nc.dram_tensor("tblB0", [NSB, ROW1], BF16)
    tblA1 = nc.dram_tensor("tblA1", [SPL, ROW1], BF16)
    tblB1 = nc.dram_tensor("tblB1", [NSB, ROW1], BF16)
    tblA2 = nc.dram_tensor("tblA2", [SPL, ROW2], BF16)
    tblB2 = nc.dram_tensor("tblB2", [NSB, ROW2], BF16)
    tabA0 = nc.dram_tensor("tabA0", [NC * SPL, ROW1], BF16, addr_space="Shared")
    tabB0 = nc.dram_tensor("tabB0", [NC * NSB, ROW1], BF16, addr_space="Shared")
    tabA1 = nc.dram_tensor("tabA1", [NC * SPL, ROW1], BF16, addr_space="Shared")
    tabB1 = nc.dram_tensor("tabB1", [NC * NSB, ROW1], BF16, addr_space="Shared")
    tabA2 = nc.dram_tensor("tabA2", [NC * SPL, ROW2], BF16, addr_space="Shared")
    tabB2 = nc.dram_tensor("tabB2", [NC * NSB, ROW2], BF16, addr_space="Shared")
    pout0 = [nc.dram_tensor(f"pout0_{k}", [512, 512], BF16) for k in range(NCHUNK)]
    pout1 = [nc.dram_tensor(f"pout1_{k}", [512, 512], BF16) for k in range(NCHUNK)]
    adsl0 = nc.dram_tensor("adsl0", [NTILE * 128, 8], BF16)
    adsl1 = nc.dram_tensor("adsl1", [NTILE * 128, 8], BF16)
    adsl2 = nc.dram_tensor("adsl2", [NTILE * 128, 8], BF16)

    rg = [list(range(NC))]
    with tile.TileContext(nc) as tc:
        with tc.tile_pool(name="cst", bufs=1) as cst, \
             tc.tile_pool(name="sb", bufs=2) as sb, \
             tc.tile_pool(name="ps", bufs=1, space="PSUM") as ps:
            io = cst.tile([128, 128], BF16)
            nc.sync.dma_start(out=io[:], in_=iota[:])
            ioc = cst.tile([128, 1], BF16)
            nc.sync.dma_start(out=ioc[:], in_=iotac[:])
            ident = cst.tile([128, 128], BF16)
            nc.vector.tensor_tensor(out=ident[:], in0=ioc[:].to_broadcast([128, 128]),
                                    in1=io[:], op=ALU.is_equal)
            b2 = cst.tile([128, 40], F32)
            nc.sync.dma_start(out=b2[:], in_=b2r[:])
            w0 = cst.tile([128, 520], BF16)
            nc.sync.dma_start(out=w0[:], in_=W0[:])
            w1 = cst.tile([128, 4 * 520], BF16)
            nc.sync.dma_start(out=w1[:], in_=W1[:])
            w2 = cst.tile([128, 4 * 268], BF16)
            nc.sync.dma_start(out=w2[:], in_=W2[:])
            sc0 = cst.tile([128, 4], F32); nc.sync.dma_start(out=sc0[:], in_=bnsc0[:])
            sh0 = cst.tile([128, 4], F32); nc.sync.dma_start(out=sh0[:], in_=bnsh0[:])
            sc1 = cst.tile([128, 4], F32); nc.sync.dma_start(out=sc1[:], in_=bnsc1[:])
            sh1 = cst.tile([128, 4], F32); nc.sync.dma_start(out=sh1[:], in_=bnsh1[:])
            idx16_all = cst.tile([128, TOT16], I16)
            nc.sync.dma_start(out=idx16_all[:], in_=eidx[:])
            dl_all = cst.tile([128, TOTDL], BF16)
            nc.sync.dma_start(out=dl_all[:], in_=edl[:])
            zz = cst.tile([128, 512], BF16)
            nc.gpsimd.memset(zz[:], 0.0)
            wregs = {w: nc.gpsimd.to_reg(w * 128)
                     for w in set(sum((_call_widths(j) for j in J1s + J2s), []))}
            # zero pad tail of last pout chunk (rows 6250.. of global layout)
            for pout in (pout0, pout1):
                r = NS - 12 * 512
                while r < 512:
                    k = min(128, 512 - r)
                    nc.sync.dma_start(out=pout[NCHUNK - 1][r:r + k, :], in_=zz[:k, :])
                    r += k
            for adsl in (adsl0, adsl1, adsl2):
                nc.sync.dma_start(out=adsl[NS:NTILE * 128, :], in_=zz[:NTILE * 128 - NS, :8])

            def ag(tbl, tab):
                return lambda: nc.gpsimd.collective_compute(
                    "AllGather", ALU.bypass, replica_groups=rg,
                    ins=[tbl[:]], outs=[tab[:]])

            def nodes(lay, pout, wt, blk, sc, sh, tblA, tblB, adsl):
                def mk(k):
                    return lambda: _node_chunk(nc, sb, ps, lay, k, None, pout,
                                               wt, blk, sc, sh, tblA, tblB, adsl)
                return {4 * k + 3: [mk(k)] for k in range(12)} | {48: [mk(12)]}

            # Layer 0 node phase (AG-A once rows<SPL done)
            for k in range(13):
                _node_chunk(nc, sb, ps, 0, k, xT, None, w0[:], 520, None, None,
                            tblA0, tblB0, adsl0)
                if k == 5:
                    ag(tblA0, tabA0)()
            ag(tblB0, tabB0)()

            il1 = nodes(1, pout0, w1[:], 520, sc0[:], sh0[:], tblA1, tblB1, adsl1)
            il1[23] = il1.get(23, []) + [ag(tblA1, tabA1)]
            il1[48] = il1.get(48, []) + [ag(tblB1, tabB1)]
            _edge_phase(nc, sb, ps, 0, meta, tabA0, tabB0, tblA0, tblB0, idx16_all,
                        dl_all, edlr, adsl0, io, ioc, ident, pout0, None, None,
                        wregs, il1)

            il2 = nodes(2, pout1, w2[:], 268, sc1[:], sh1[:], tblA2, tblB2, adsl2)
            il2[23] = il2.get(23, []) + [ag(tblA2, tabA2)]
            il2[48] = il2.get(48, []) + [ag(tblB2, tabB2)]
            _edge_phase(nc, sb, ps, 1, meta, tabA1, tabB1, tblA1, tblB1, idx16_all,
                        dl_all, edlr, adsl1, io, ioc, ident, pout1, None, None,
                        wregs, il2)

            _edge_phase(nc, sb, ps, 2, meta, tabA2, tabB2, tblA2, tblB2, idx16_all,
                        dl_all, edlr, adsl2, io, ioc, ident, None, out, b2[:], wregs)
    _hoist_waits(nc)
    mask = {}
    for lib in all_libraries:
        for ty in lib.instructions:
            mask[ty] = mask.get(ty, 0) | (1 << lib.index)
    bass_rust.insert_library_loads(nc, mask, len(all_libraries), standard.index)
    mybir.codegen_inst_isa_subclasses(nc)
    return nc


def _prep_edges(edge_index):
    """Per-core edge arrays for dma_gather (self-loops excluded).

    Returns (meta, per-core (idx16, dl, dlr) arrays) where meta =
    (J1s, J2s, off16, offdl, offdlr, J1M, J2M); offsets have NTILE+1
    entries (element counts into the flat arrays).
    """
    src = edge_index[0].astype(np.int64)
    dst = edge_index[1].astype(np.int64)
    core = dst // NS
    tloc = (dst % NS) // 128
    s_loc = src % NS
    s_core = src // NS
    inA = s_loc < SPL
    rowA = s_core * SPL + s_loc
    rowB = s_core * NSB + (s_loc - SPL)
    percore = []
    for c in range(NC):
        m = core == c
        d_c, t_c = dst[m], tloc[m]
        a_c, ra_c, rb_c = inA[m], rowA[m], rowB[m]
        dl_c = (d_c % NS) - t_c * 128
        tiles = []
        for t in range(NTILE):
            mt = t_c == t
            aa = a_c[mt]
            tiles.append((ra_c[mt][aa], dl_c[mt][aa],
                          rb_c[mt][~aa], dl_c[mt][~aa]))
        percore.append(tiles)
    J1s, J2s = [], []
    for t in range(NTILE):
        J1s.append(max(-(-len(percore[c][t][0]) // 128) for c in range(NC)))
        J2s.append(max(-(-len(percore[c][t][2]) // 128) for c in range(NC)))
    J1M, J2M = max(J1s), max(J2s)
    off16 = [0]; offdl = [0]; offdlr = [0]
    for t in range(NTILE):
        ns = J1s[t] + J2s[t]
        off16.append(off16[-1] + ns * 8)
        offdl.append(offdl[-1] + ns)
        offdlr.append(offdlr[-1] + ns * 128)
    meta = (J1s, J2s, off16, offdl, offdlr, J1M, J2M)
    arrs = []
    for c in range(NC):
        idx16 = np.zeros((16, off16[-1]), np.int16)
        dlf = np.full((128, offdl[-1]), 999.0, np.float32)
        dlrf = np.full((1, offdlr[-1]), 999.0, np.float32)
        for t in range(NTILE):
            rA, dA, rB, dB = percore[c][t]
            for (rr, dd, joff) in ((rA, dA, 0), (rB, dB, J1s[t])):
                e = np.arange(len(rr))
                idx16[e % 16, off16[t] + joff * 8 + e // 16] = rr.astype(np.int16)
                dlf[e % 128, offdl[t] + joff + e // 128] = dd
                dlrf[0, offdlr[t] + joff * 128 + (e // 128) * 128 + e % 128] = dd
        arrs.append((np.tile(idx16, (8, 1)), dlf.astype(bf), dlrf.astype(bf)))
    return meta, arrs


def kernel(x, edge_index, W0, as0, ad0, b0, g0, bt0, m0, v0,
           W1, as1, ad1, b1, g1, bt1, m1, v1,
           W2, as2, ad2, b2):
    x = np.asarray(x, np.float32)
    meta, earrs = _prep_edges(np.asarray(edge_index))

    # host-side packing
    def packW(W, a_s, a_d, d_out_pad, Hh, Cc, headpad):
        # W: [d_in, d_out]; returns [128, NFB*(d_out_pad + 2H)] bf16
        d_in = W.shape[0]
        NFB = d_in // 128
        if headpad:  # L2: pad head layout (3 heads of 40 per 128-block)
            Wp = np.zeros((d_in, 256), np.float32)
            Wp[:, 0:120] = W[:, 0:120]
            Wp[:, 128:248] = W[:, 120:240]
        else:
            Wp = W.astype(np.float32)
        Wa = np.zeros((d_in, 2 * Hh), np.float32)
        for h in range(Hh):
            Wa[:, 2 * h] = W[:, h * Cc:(h + 1) * Cc] @ a_s[h]
            Wa[:, 2 * h + 1] = W[:, h * Cc:(h + 1) * Cc] @ a_d[h]
        blk = np.concatenate([Wp, Wa], axis=1)          # [d_in, d_out_pad+2H]
        blk = blk.reshape(NFB, 128, -1)
        return np.concatenate([blk[i] for i in range(NFB)], axis=1).astype(bf)

    W0 = np.asarray(W0, np.float32); W1 = np.asarray(W1, np.float32); W2 = np.asarray(W2, np.float32)
    as0 = np.asarray(as0, np.float32); ad0 = np.asarray(ad0, np.float32)
    as1 = np.asarray(as1, np.float32); ad1 = np.asarray(ad1, np.float32)
    as2 = np.asarray(as2, np.float32); ad2 = np.asarray(ad2, np.float32)
    W0p = packW(W0, as0, ad0, 512, 4, 128, False)
    W1p = packW(W1, as1, ad1, 512, 4, 128, False)
    W2p = packW(W2, as2, ad2, 256, 6, 40, True)

    def bnfold(g, bt, m, v, b):
        sc = np.asarray(g, np.float32) / np.sqrt(np.asarray(v, np.float32) + BN_EPS)
        sh = (np.asarray(b, np.float32) - np.asarray(m, np.float32)) * sc + np.asarray(bt, np.float32)
        return sc.reshape(4, 128).T.copy(), sh.reshape(4, 128).T.copy()
    sc0, sh0 = bnfold(g0, bt0, m0, v0, b0)
    sc1, sh1 = bnfold(g1, bt1, m1, v1, b1)
    b2rep = np.broadcast_to(np.asarray(b2, np.float32), (128, 40)).copy()

    iota = np.broadcast_to(np.arange(128, dtype=np.float32), (128, 128)).astype(bf)
    iotac = np.arange(128, dtype=np.float32)[:, None].astype(bf)

    xT = x.T.astype(bf)   # [128, N]
    in_maps = []
    for c in range(NC):
        xts = np.zeros((128, PADN), bf)
        xts[:, :NS] = xT[:, c * NS:(c + 1) * NS]
        idx16, dlf, dlrf = earrs[c]
        in_maps.append({
            "xT": xts, "W0p": W0p, "W1p": W1p, "W2p": W2p,
            "bnsc0": sc0, "bnsh0": sh0, "bnsc1": sc1, "bnsh1": sh1,
            "b2r": b2rep, "iota": iota, "iotac": iotac,
            "eidx": idx16, "edl": dlf, "edlr": dlrf,
        })

    global _last_in_maps, _last_meta
    _last_in_maps = in_maps
    _last_meta = meta
    nc = _build(meta)
    res = run_bass_kernel_spmd(nc, in_maps, core_ids=list(range(NC)))
    out = np.concatenate([res.results[c]["out"] for c in range(NC)], axis=0)
    return out.astype(np.float32)
